# revision 1
# baseline (speedup 1.0000x reference)
"""Bass/Tile TRN2 kernel for nn_ConvTran_618475290811.

ConvTran tiny transformer: conv embed + BN + GELU + tAPE + eRPE attention
(bias added AFTER softmax) + FFN + mean-pool + classifier head.
B=8 batch elements, data-parallel one per NeuronCore (8 cores).

Key tricks:
 - attention computed in transposed (S^T = [keys, queries]) layout; softmax
   denominator produced for free via a ones-column appended to V.
 - no division for softmax: LayerNorm scale-invariance lets us feed
   z = exp@v + denom * (R@v) into the to_out LayerNorm.
 - eRPE Toeplitz bias R@v done with 15 diagonal-block stationary weights per
   head (gathered with one negative-stride DMA per head from the bias table).
 - fp32r matmuls (full PE rate at N>=512) with fp32 storage.
"""
import math
import numpy as np

import concourse.bass as bass
import concourse.bacc as bacc
import concourse.tile as tile
from concourse import mybir

B, L, E, H, NCls, DFF, KW = 8, 1024, 24, 8, 10, 256, 8
HD = E // H  # 3
NCORES = 8
F32 = mybir.dt.float32
BF16 = mybir.dt.bfloat16
R32 = mybir.dt.float32r
AF = mybir.ActivationFunctionType
OP = mybir.AluOpType
SCALE = float(E) ** -0.5
INV_SQRT2 = 0.7071067811865476
EPS = 1e-5


def _ap(t, off, pattern):
    return bass.AP(t, off, pattern)


def build_nc(erf_func=AF.Erf):
    nc = bacc.Bacc("TRN2", target_bir_lowering=False, debug=False)

    # ---- DRAM I/O ----
    d_xpad = nc.dram_tensor("xpad", [L + KW - 1], F32, kind="ExternalInput")
    d_cw = nc.dram_tensor("cw", [KW, E], F32, kind="ExternalInput")
    d_cb = nc.dram_tensor("cb", [E, 1], F32, kind="ExternalInput")
    d_cberf = nc.dram_tensor("cberf", [E, 1], F32, kind="ExternalInput")
    d_peT = nc.dram_tensor("peT", [E, L], F32, kind="ExternalInput")
    d_wq = nc.dram_tensor("wq", [E, E], F32, kind="ExternalInput")
    d_wk = nc.dram_tensor("wk", [E, E], F32, kind="ExternalInput")
    d_wv = nc.dram_tensor("wv", [E, E], F32, kind="ExternalInput")
    d_relT = nc.dram_tensor("relT", [H, 2 * L - 1], BF16, kind="ExternalInput")
    d_w1 = nc.dram_tensor("w1", [E, DFF], F32, kind="ExternalInput")
    d_b1c = nc.dram_tensor("b1c", [128, 2], F32, kind="ExternalInput")
    d_w2 = nc.dram_tensor("w2", [128, 2, E], F32, kind="ExternalInput")
    d_b2 = nc.dram_tensor("b2", [E, 1], F32, kind="ExternalInput")
    d_ow = nc.dram_tensor("ow", [E, NCls], F32, kind="ExternalInput")
    d_ob = nc.dram_tensor("ob", [NCls, 1], F32, kind="ExternalInput")
    d_ident = nc.dram_tensor("ident", [128, 128], F32, kind="ExternalInput")
    # 6 LayerNorm gain/bias rows: attn_ln, ln1, ln2
    d_lng = nc.dram_tensor("lng", [3, E], F32, kind="ExternalInput")
    d_lnb = nc.dram_tensor("lnb", [3, E], F32, kind="ExternalInput")
    d_out = nc.dram_tensor("out", [NCls, 1], F32, kind="ExternalOutput")

    with tile.TileContext(nc) as tc:
        _emit(tc, nc, erf_func, d_xpad, d_cw, d_cb, d_cberf, d_peT, d_wq,
              d_wk, d_wv, d_relT, d_w1, d_b1c, d_w2, d_b2, d_ow, d_ob,
              d_ident, d_lng, d_lnb, d_out)
    nc.compile()
    return nc


def _layernorm(nc, pool, x, out, g_bc, b_bc, eps_sb, pfx):
    """LN over last dim (24) of x [128, 8, 24] -> out [128, 8, 24]."""
    sums = pool.tile([128, 8], F32, name=f"{pfx}_sums", tag="ln_sums")
    nc.vector.tensor_reduce(sums, x, axis=mybir.AxisListType.X, op=OP.add)
    mean = pool.tile([128, 8], F32, name=f"{pfx}_mean", tag="ln_mean")
    nc.vector.tensor_scalar_mul(mean, sums, 1.0 / E)
    meanb = _ap(mean.tensor, mean.offset, [mean.ap[0], list(mean.ap[1]), [0, E]])
    cent = pool.tile([128, 8, E], F32, name=f"{pfx}_cent", tag="ln_cent")
    nc.vector.tensor_tensor(cent, x, meanb, OP.subtract)
    sq = pool.tile([128, 8, E], F32, name=f"{pfx}_sq", tag="ln_sq")
    nc.vector.tensor_tensor(sq, cent, cent, OP.mult)
    sqs = pool.tile([128, 8], F32, name=f"{pfx}_sqs", tag="ln_sqs")
    nc.vector.tensor_reduce(sqs, sq, axis=mybir.AxisListType.X, op=OP.add)
    var = pool.tile([128, 8], F32, name=f"{pfx}_var", tag="ln_var")
    nc.vector.tensor_scalar_mul(var, sqs, 1.0 / E)
    lnv = pool.tile([128, 8], F32, name=f"{pfx}_lnv", tag="ln_lnv")
    nc.scalar.activation(lnv, var, AF.Ln, bias=eps_sb, scale=1.0)
    rstd = pool.tile([128, 8], F32, name=f"{pfx}_rstd", tag="ln_rstd")
    nc.scalar.activation(rstd, lnv, AF.Exp, scale=-0.5)
    rstdb = _ap(rstd.tensor, rstd.offset, [rstd.ap[0], list(rstd.ap[1]), [0, E]])
    nrm = pool.tile([128, 8, E], F32, name=f"{pfx}_nrm", tag="ln_nrm")
    nc.vector.tensor_tensor(nrm, cent, rstdb, OP.mult)
    # apply g (broadcast over partitions and lt): g_bc is [128, 24]
    gv = _ap(g_bc.tensor, g_bc.offset, [g_bc.ap[0], [0, 8], list(g_bc.ap[1])])
    bv = _ap(b_bc.tensor, b_bc.offset, [b_bc.ap[0], [0, 8], list(b_bc.ap[1])])
    nc.vector.tensor_tensor(nrm, nrm, gv, OP.mult)
    nc.vector.tensor_tensor(out, nrm, bv, OP.add)


def _emit(tc, nc, erf_func, d_xpad, d_cw, d_cb, d_cberf, d_peT, d_wq, d_wk,
          d_wv, d_relT, d_w1, d_b1c, d_w2, d_b2, d_ow, d_ob, d_ident,
          d_lng, d_lnb, d_out):
    from contextlib import ExitStack
    ctx = ExitStack()
    with ctx:
        singles = ctx.enter_context(tc.tile_pool(name="singles", bufs=1))
        texp_pool = ctx.enter_context(tc.tile_pool(name="texp", bufs=3))
        u_pool = ctx.enter_context(tc.tile_pool(name="upool", bufs=2))
        aosb_pool = ctx.enter_context(tc.tile_pool(name="aosb", bufs=2))
        scratch = ctx.enter_context(tc.tile_pool(name="scratch", bufs=1))

        # ---- load params ----
        cw = singles.tile([KW, E], F32, name="cw_sb")
        nc.sync.dma_start(out=cw, in_=d_cw.ap())
        cb = singles.tile([E, 1], F32, name="cb_sb")
        nc.sync.dma_start(out=cb, in_=d_cb.ap())
        cberf = singles.tile([E, 1], F32, name="cberf_sb")
        nc.sync.dma_start(out=cberf, in_=d_cberf.ap())
        peT = singles.tile([E, L], F32, name="peT_sb")
        nc.sync.dma_start(out=peT, in_=d_peT.ap())
        wq = singles.tile([E, E], F32, name="wq_sb")
        nc.sync.dma_start(out=wq, in_=d_wq.ap())
        wk = singles.tile([E, E], F32, name="wk_sb")
        nc.sync.dma_start(out=wk, in_=d_wk.ap())
        wv = singles.tile([E, E], F32, name="wv_sb")
        nc.sync.dma_start(out=wv, in_=d_wv.ap())
        w1 = singles.tile([E, DFF], F32, name="w1_sb")
        nc.sync.dma_start(out=w1, in_=d_w1.ap())
        b1c = singles.tile([128, 2], F32, name="b1c_sb")
        nc.sync.dma_start(out=b1c, in_=d_b1c.ap())
        w2 = singles.tile([128, 2, E], F32, name="w2_sb")
        nc.sync.dma_start(out=w2, in_=d_w2.ap())
        b2 = singles.tile([E, 1], F32, name="b2_sb")
        nc.sync.dma_start(out=b2, in_=d_b2.ap())
        ow = singles.tile([E, NCls], F32, name="ow_sb")
        nc.sync.dma_start(out=ow, in_=d_ow.ap())
        ob = singles.tile([NCls, 1], F32, name="ob_sb")
        nc.sync.dma_start(out=ob, in_=d_ob.ap())
        ident = singles.tile([128, 128], F32, name="ident_sb")
        nc.sync.dma_start(out=ident, in_=d_ident.ap())
        # LN g/b broadcast tiles [128, 24] each
        lng_bc = singles.tile([128, 3, E], F32, name="lng_bc")
        nc.sync.dma_start(out=lng_bc,
                          in_=_ap(d_lng, 0, [[0, 128], [E, 3], [1, E]]))
        lnb_bc = singles.tile([128, 3, E], F32, name="lnb_bc")
        nc.sync.dma_start(out=lnb_bc,
                          in_=_ap(d_lnb, 0, [[0, 128], [E, 3], [1, E]]))
        eps_sb = singles.tile([128, 1], F32, name="eps_sb")
        nc.vector.memset(eps_sb, EPS)
        # dependency-free dummy activations: preload ACT table sets so real
        # activation instructions don't carry the table-load waits
        dummy_act = singles.tile([1, 1], F32, name="dummy_act")
        nc.vector.memset(dummy_act, 0.5)
        nc.scalar.activation(dummy_act, dummy_act, AF.Exp, scale=1.0)
        nc.scalar.activation(dummy_act, dummy_act, AF.Ln, scale=1.0)
        nc.scalar.activation(dummy_act, dummy_act, erf_func, scale=1.0)
        ones128 = singles.tile([128, 1], F32, name="ones128")
        nc.vector.memset(ones128, 1.0)
        ones11 = singles.tile([1, 1], F32, name="ones11")
        nc.vector.memset(ones11, 1.0)
        z1 = singles.tile([1, 128], F32, name="z1_sb")
        nc.vector.memset(z1, 0.0)
        z2 = singles.tile([1, 192], F32, name="z2_sb")
        nc.vector.memset(z2, 0.0)

        # big single tiles
        V_sb = singles.tile([128, 8, 8, 4], BF16, name="V_sb")
        q_all = singles.tile([HD, H, L], BF16, name="q_all")
        k_all = singles.tile([HD, H, L], BF16, name="k_all")
        aoT_stack = singles.tile([32, L], F32, name="aoT_stack")
        xsrcT = singles.tile([E, L], F32, name="xsrcT")
        xposT = singles.tile([E, L], F32, name="xposT")

        # ============ phase 1: conv embed + BN + GELU + tAPE ============
        with tc.tile_pool(name="ph1ps", bufs=1, space="PSUM") as ph1ps, \
             tc.tile_pool(name="ph1sb", bufs=1) as ph1sb:
            xcol = ph1sb.tile([KW, L], F32, name="xcol")
            nc.sync.dma_start(out=xcol, in_=_ap(d_xpad, 0, [[1, KW], [1, L]]))
            conv_ps = ph1ps.tile([E, L], F32, name="conv_ps")
            for hh in range(2):
                nc.tensor.matmul(conv_ps[:, hh * 512:(hh + 1) * 512],
                                 cw,
                                 xcol[:, hh * 512:(hh + 1) * 512],
                                 start=True, stop=True)
            # exact GELU via erf: gelu(y) = y * (0.5 + 0.5*erf(y/sqrt(2)))
            e_t = ph1sb.tile([E, L], F32, name="e_t")
            nc.scalar.activation(e_t, conv_ps, erf_func, bias=cberf,
                                 scale=INV_SQRT2)
            y_t = ph1sb.tile([E, L], F32, name="y_t")
            nc.scalar.activation(y_t, conv_ps, AF.Identity, bias=cb, scale=1.0)
            t05 = ph1sb.tile([E, L], F32, name="t05")
            nc.vector.tensor_scalar(t05, e_t, 0.5, 0.5, OP.mult, OP.add)
            nc.vector.tensor_tensor(xsrcT, y_t, t05, OP.mult)
            nc.vector.tensor_tensor(xposT, xsrcT, peT, OP.add)

            # ---- Q^T, K^T projections + per-head repack ----
            for (w_, dst, nm) in ((wq, q_all, "q"), (wk, k_all, "k")):
                prj = ph1ps.tile([E, L], F32, name=f"prj_{nm}", tag="prj")
                for hh in range(2):
                    nc.tensor.matmul(prj[:, hh * 512:(hh + 1) * 512],
                                     w_,
                                     xposT[:, hh * 512:(hh + 1) * 512],
                                     start=True, stop=True)
                stg = ph1sb.tile([E, L], BF16, name=f"stg_{nm}", tag="stg")
                nc.vector.tensor_copy(stg, prj)
                for h in range(H):
                    nc.sync.dma_start(out=dst[:, h, :],
                                      in_=stg[3 * h:3 * h + 3, :])

            # ---- V in [L, head, dim|1] layout ----
            nc.vector.memset(V_sb, 1.0)
            for jt in range(8):
                vps = ph1ps.tile([128, E], F32, name=f"vps{jt}", tag="vps")
                nc.tensor.matmul(vps,
                                 xposT[:, jt * 128:(jt + 1) * 128],
                                 wv, start=True, stop=True)
                vview = _ap(vps.tensor, vps.offset, [vps.ap[0], [3, 8], [1, 3]])
                dst = _ap(V_sb.tensor, V_sb.offset + jt * 32,
                          [V_sb.ap[0], [4, 8], [1, 3]])
                nc.vector.tensor_copy(dst, vview)

        # ============ phase 2: attention ============
        with tc.tile_pool(name="biasps", bufs=1, space="PSUM") as biasps:
            bias_ps = biasps.tile([128, H, 8, HD], F32, name="bias_ps")
            flat = bias_ps.rearrange("p a b c -> p (a b c)")
            nc.tensor.matmul(flat, z1, z2, start=True, stop=False,
                             skip_group_check=True)

            with tc.tile_pool(name="sps", bufs=2, space="PSUM") as sps, \
                 tc.tile_pool(name="aops", bufs=1, space="PSUM") as aops:
                for h in range(H):
                    u_t = u_pool.tile([128, 15 * 128], BF16, name=f"u{h}",
                                      tag="u")
                    # relT is host-flipped: relT[h, m] = table[2046 - m, h];
                    # U[j', c'] = t_h[c' - j' + 127] -> flipped idx 1919+j'-c'
                    nc.sync.dma_start(
                        out=u_t,
                        in_=_ap(d_relT, h * (2 * L - 1) + 1919,
                                [[1, 128], [-1, 15 * 128]]))
                    ao_ps = aops.tile([4, L], F32, name=f"ao{h}", tag="ao")
                    for jt in range(8):
                        s_ps = sps.tile([128, L], F32, name=f"s{h}_{jt}",
                                        tag="s")
                        lw = k_all[:, h, jt * 128:(jt + 1) * 128]
                        for hh in range(2):
                            nc.tensor.matmul(
                                s_ps[:, hh * 512:(hh + 1) * 512], lw,
                                q_all[:, h, hh * 512:(hh + 1) * 512],
                                start=True, stop=True)
                        texp = texp_pool.tile([128, L], BF16,
                                              name=f"texp{h}_{jt}", tag="texp")
                        nc.scalar.activation(texp, s_ps, AF.Exp, scale=SCALE)
                        v1 = V_sb[:, jt, h, :]
                        for hh in range(2):
                            nc.tensor.matmul(
                                ao_ps[:, hh * 512:(hh + 1) * 512],
                                v1,
                                texp[:, hh * 512:(hh + 1) * 512],
                                start=(jt == 0), stop=(jt == 7))
                    # eRPE Toeplitz bias: 15 diagonal blocks
                    for d in range(-7, 8):
                        jt0 = max(0, -d)
                        n = 8 - abs(d)
                        it0 = max(0, d)
                        nc.tensor.matmul(
                            bias_ps[:, h, it0:it0 + n, :],
                            u_t[:, (d + 7) * 128:(d + 8) * 128],
                            V_sb[:, jt0:jt0 + n, h, 0:3],
                            start=False, stop=False, skip_group_check=True)
                    ao_sb = aosb_pool.tile([4, L], F32, name=f"aosb{h}",
                                           tag="aosb")
                    nc.vector.tensor_copy(ao_sb, ao_ps)
                    nc.sync.dma_start(out=aoT_stack[4 * h:4 * h + 4, :],
                                      in_=ao_sb)
                nc.tensor.matmul(flat, z1, z2, start=False, stop=True,
                                 skip_group_check=True)

            # ======== phase 3: transpose ao + z assembly ========
            z_sb = singles.tile([128, 8, E], F32, name="z_sb")
            with tc.tile_pool(name="trps", bufs=2, space="PSUM") as trps, \
                 tc.tile_pool(name="trsb", bufs=2) as trsb:
                for lt in range(8):
                    tr_ps = trps.tile([128, 32], F32, name=f"tr{lt}", tag="tr")
                    nc.tensor.transpose(tr_ps,
                                        aoT_stack[:, lt * 128:(lt + 1) * 128],
                                        ident[:32, :32])
                    tr_sb = trsb.tile([128, 8, 4], F32, name=f"trsb{lt}",
                                      tag="trs")
                    nc.vector.tensor_copy(tr_sb, tr_ps)
                    # ao = A * (1/d) + B  (d = denom col 3; B = bias_ps slice)
                    rec = trsb.tile([128, 8], F32, name=f"rec{lt}", tag="rec")
                    nc.vector.reciprocal(rec, tr_sb[:, :, 3])
                    recb = _ap(rec.tensor, rec.offset,
                               [rec.ap[0], list(rec.ap[1]), [0, 3]])
                    an = trsb.tile([128, 8, 3], F32, name=f"an{lt}", tag="an")
                    nc.vector.tensor_tensor(an, tr_sb[:, :, 0:3], recb,
                                            OP.mult)
                    nc.vector.tensor_tensor(z_sb[:, lt, :].rearrange(
                        "p (a b) -> p a b", a=8), an, bias_ps[:, :, lt, :],
                        OP.add)

        # ======== phase 4: LNs + FFN + pool + head ========
        y1 = singles.tile([128, 8, E], F32, name="y1_sb")
        att_L = singles.tile([128, 8, E], F32, name="attL_sb")
        y2 = singles.tile([128, 8, E], F32, name="y2_sb")
        out_L = singles.tile([128, 8, E], F32, name="outL_sb")
        zln = singles.tile([128, 8, E], F32, name="zln_sb")
        attT = singles.tile([E, L], F32, name="attT_sb")
        ffh0 = singles.tile([128, L], F32, name="ffh0_sb")
        ffh1 = singles.tile([128, L], F32, name="ffh1_sb")
        ffT = singles.tile([E, L], F32, name="ffT_sb")

        _layernorm(nc, scratch, z_sb, zln, lng_bc[:, 0, :], lnb_bc[:, 0, :],
                   eps_sb, "aln")
        with tc.tile_pool(name="xsps", bufs=2, space="PSUM") as xsps:
            for lt in range(8):
                xs_ps = xsps.tile([128, E], F32, name=f"xs{lt}", tag="xs")
                nc.tensor.transpose(xs_ps, xsrcT[:, lt * 128:(lt + 1) * 128],
                                    ident[:E, :E])
                nc.vector.tensor_tensor(y1[:, lt, :], zln[:, lt, :], xs_ps,
                                        OP.add)
        _layernorm(nc, scratch, y1, att_L, lng_bc[:, 1, :], lnb_bc[:, 1, :],
                   eps_sb, "ln1")

        with tc.tile_pool(name="atps", bufs=1, space="PSUM") as atps:
            attT_ps = atps.tile([E, L], F32, name="attT_ps")
            for lt in range(8):
                nc.tensor.transpose(attT_ps[:, lt * 128:(lt + 1) * 128],
                                    att_L[:, lt, :], ident)
            nc.vector.tensor_copy(attT, attT_ps)

        with tc.tile_pool(name="ffps", bufs=2, space="PSUM") as ffps:
            for p2, ffh in ((0, ffh0), (1, ffh1)):
                ffh_ps = ffps.tile([128, L], F32, name=f"ffh{p2}", tag="ffh")
                for hh in range(2):
                    nc.tensor.matmul(ffh_ps[:, hh * 512:(hh + 1) * 512],
                                     w1[:, p2 * 128:(p2 + 1) * 128],
                                     attT[:, hh * 512:(hh + 1) * 512],
                                     start=True, stop=True)
                nc.scalar.activation(ffh, ffh_ps, AF.Relu,
                                     bias=b1c[:, p2:p2 + 1], scale=1.0)

        with tc.tile_pool(name="f2ps", bufs=1, space="PSUM") as f2ps:
            ffT_ps = f2ps.tile([E, L], F32, name="ffT_ps")
            for hh in range(2):
                for p2, ffh in ((0, ffh0), (1, ffh1)):
                    nc.tensor.matmul(
                        ffT_ps[:, hh * 512:(hh + 1) * 512],
                        w2[:, p2, :],
                        ffh[:, hh * 512:(hh + 1) * 512],
                        start=(p2 == 0), stop=(p2 == 1))
            nc.scalar.activation(ffT, ffT_ps, AF.Identity, bias=b2, scale=1.0)

        with tc.tile_pool(name="fmps", bufs=2, space="PSUM") as fmps:
            for lt in range(8):
                ff_ps = fmps.tile([128, E], F32, name=f"ffm{lt}", tag="ffm")
                nc.tensor.transpose(ff_ps, ffT[:, lt * 128:(lt + 1) * 128],
                                    ident[:E, :E])
                nc.vector.tensor_tensor(y2[:, lt, :], att_L[:, lt, :], ff_ps,
                                        OP.add)
        _layernorm(nc, scratch, y2, out_L, lng_bc[:, 2, :], lnb_bc[:, 2, :],
                   eps_sb, "ln2")

        with tc.tile_pool(name="hdps", bufs=1, space="PSUM") as hdps, \
             tc.tile_pool(name="hdsb", bufs=1) as hdsb:
            pooled_ps = hdps.tile([1, E], F32, name="pooled_ps")
            for lt in range(8):
                nc.tensor.matmul(pooled_ps, ones128, out_L[:, lt, :],
                                 start=(lt == 0), stop=(lt == 7))
            pooled_sb = hdsb.tile([1, E], F32, name="pooled_sb")
            nc.vector.tensor_copy(pooled_sb, pooled_ps)
            pooledT_ps = hdps.tile([E, 1], F32, name="pooledT_ps")
            nc.tensor.matmul(pooledT_ps, pooled_sb, ones11, start=True,
                             stop=True)
            pooledT_sb = hdsb.tile([E, 1], F32, name="pooledT_sb")
            nc.vector.tensor_copy(pooledT_sb, pooledT_ps)
            logits_ps = hdps.tile([NCls, 1], F32, name="logits_ps")
            nc.tensor.matmul(logits_ps, ow, pooledT_sb, start=True, stop=True)
            logits_sb = hdsb.tile([NCls, 1], F32, name="logits_sb")
            nc.scalar.activation(logits_sb, logits_ps, AF.Identity, bias=ob,
                                 scale=1.0 / L)
            nc.sync.dma_start(out=d_out.ap(), in_=logits_sb)


def host_prep(inputs, erf=None):
    """Host-side parameter prep (tiny, O(E*K)). Returns (shared, per_core)."""
    f32 = np.float32
    a = (inputs["bn_gamma"] / np.sqrt(inputs["bn_var"] + EPS)).astype(f32)
    cw = (inputs["conv_w"][:, 0, :].T * a[None, :]).astype(f32)  # [K, E]
    cb = ((inputs["conv_b"] - inputs["bn_mean"]) * a
          + inputs["bn_beta"]).astype(f32).reshape(E, 1)
    # tAPE positional encoding
    pos = np.arange(L, dtype=f32)[:, None]
    div = np.exp(np.arange(0, E, 2, dtype=f32) * (-math.log(10000.0) / E))
    ang = pos * div * (float(E) / float(L))
    pe = np.zeros((L, E), f32)
    pe[:, 0::2] = np.sin(ang)
    pe[:, 1::2] = np.cos(ang)
    b1 = inputs["ff_b1"].astype(f32)
    b1c = np.stack([b1[:128], b1[128:]], axis=1)  # [128, 2]
    shared = {
        "cw": cw,
        "cb": cb,
        "cberf": (cb * INV_SQRT2).astype(f32),
        "peT": pe.T.copy(),
        "wq": inputs["wq"].astype(f32),
        "wk": inputs["wk"].astype(f32),
        "wv": inputs["wv"].astype(f32),
        "relT": np.ascontiguousarray(
            inputs["rel_bias_table"].T.astype(f32)[:, ::-1]).astype(
                mybir.dt.np(BF16)),  # [H, 2047] flipped, bf16
        "w1": inputs["ff_w1"].astype(f32),
        "b1c": b1c.copy(),
        "w2": np.ascontiguousarray(
            inputs["ff_w2"].astype(f32).reshape(2, 128, E).transpose(1, 0, 2)),
        "b2": inputs["ff_b2"].astype(f32).reshape(E, 1),
        "ow": inputs["out_w"].astype(f32),
        "ob": inputs["out_b"].astype(f32).reshape(NCls, 1),
        "ident": np.eye(128, dtype=f32),
        "lng": np.stack([inputs["attn_ln_g"], inputs["ln1_g"],
                         inputs["ln2_g"]]).astype(f32),
        "lnb": np.stack([inputs["attn_ln_b"], inputs["ln1_b"],
                         inputs["ln2_b"]]).astype(f32),
    }
    x = inputs["x"].astype(f32)  # (B, 1, L)
    per_core = []
    for b in range(B):
        xpad = np.zeros((L + KW - 1,), f32)
        xpad[3:3 + L] = x[b, 0]
        per_core.append({"xpad": xpad, **shared})
    return per_core


_NC_CACHE = {}


def kernel(**inputs) -> np.ndarray:
    from concourse.bass_utils import run_bass_kernel_spmd
    if "nc" not in _NC_CACHE:
        _NC_CACHE["nc"] = build_nc()
    nc = _NC_CACHE["nc"]
    in_maps = host_prep(inputs)
    res = run_bass_kernel_spmd(nc, in_maps, core_ids=list(range(NCORES)))
    out = np.stack([res.results[b]["out"].reshape(NCls) for b in range(B)])
    return out.astype(np.float32)


if __name__ == "__main__":
    import reference
    ins = {k: np.asarray(v) for k, v in reference.setup_inputs().items()}
    got = kernel(**ins)
    exp = np.asarray(reference.reference(**reference.setup_inputs()))
    err = np.abs(got - exp).max() / np.abs(exp).max()
    print("Relative error:", err)



# revision 8
# speedup vs baseline: 8.9388x; 8.9388x over previous
"""Bass/Tile TRN2 kernel for nn_ConvTran_618475290811.

ConvTran tiny transformer: conv embed + BN + GELU + tAPE + eRPE attention
(bias added AFTER softmax) + FFN + mean-pool + classifier head.
B=8 batch elements, data-parallel one per NeuronCore (8 cores).

Key tricks:
 - attention computed in transposed (S^T = [keys, queries]) layout; softmax
   denominator produced for free via a ones-column appended to V.
 - no division for softmax: LayerNorm scale-invariance lets us feed
   z = exp@v + denom * (R@v) into the to_out LayerNorm.
 - eRPE Toeplitz bias R@v done with 15 diagonal-block stationary weights per
   head (gathered with one negative-stride DMA per head from the bias table).
 - fp32r matmuls (full PE rate at N>=512) with fp32 storage.
"""
import math
import numpy as np

import concourse.bass as bass
import concourse.bacc as bacc
import concourse.tile as tile
from concourse import mybir

B, L, E, H, NCls, DFF, KW = 8, 1024, 24, 8, 10, 256, 8
HD = E // H  # 3
NCORES = 8
F32 = mybir.dt.float32
BF16 = mybir.dt.bfloat16
R32 = mybir.dt.float32r
AF = mybir.ActivationFunctionType
OP = mybir.AluOpType
SCALE = float(E) ** -0.5
INV_SQRT2 = 0.7071067811865476
EPS = 1e-5


def _ap(t, off, pattern):
    return bass.AP(t, off, pattern)


def build_nc(erf_func=AF.Erf):
    nc = bacc.Bacc("TRN2", target_bir_lowering=False, debug=False)

    # ---- DRAM I/O ----
    d_xpad = nc.dram_tensor("xpad", [L + KW - 1], F32, kind="ExternalInput")
    d_cw = nc.dram_tensor("cw", [KW, E], F32, kind="ExternalInput")
    d_cb = nc.dram_tensor("cb", [E, 1], F32, kind="ExternalInput")
    d_cberf = nc.dram_tensor("cberf", [E, 1], F32, kind="ExternalInput")
    d_peT = nc.dram_tensor("peT", [E, L], F32, kind="ExternalInput")
    d_wq = nc.dram_tensor("wq", [E, E], F32, kind="ExternalInput")
    d_wk = nc.dram_tensor("wk", [E, E], F32, kind="ExternalInput")
    d_wv = nc.dram_tensor("wv", [E, E], F32, kind="ExternalInput")
    d_relU = nc.dram_tensor("relU", [128, H, 15 * 128], BF16,
                            kind="ExternalInput")
    d_w1 = nc.dram_tensor("w1", [E, DFF], F32, kind="ExternalInput")
    d_b1c = nc.dram_tensor("b1c", [128, 2], F32, kind="ExternalInput")
    d_w2 = nc.dram_tensor("w2", [128, 2, E], F32, kind="ExternalInput")
    d_b2 = nc.dram_tensor("b2", [E, 1], F32, kind="ExternalInput")
    d_ow = nc.dram_tensor("ow", [E, NCls], F32, kind="ExternalInput")
    d_ob = nc.dram_tensor("ob", [NCls, 1], F32, kind="ExternalInput")
    d_ident = nc.dram_tensor("ident", [128, 128], F32, kind="ExternalInput")
    # 6 LayerNorm gain/bias rows: attn_ln, ln1, ln2
    d_lng = nc.dram_tensor("lng", [3, E], F32, kind="ExternalInput")
    d_lnb = nc.dram_tensor("lnb", [3, E], F32, kind="ExternalInput")
    d_out = nc.dram_tensor("out", [NCls, 1], F32, kind="ExternalOutput")

    with tile.TileContext(nc) as tc:
        _emit(tc, nc, erf_func, d_xpad, d_cw, d_cb, d_cberf, d_peT, d_wq,
              d_wk, d_wv, d_relU, d_w1, d_b1c, d_w2, d_b2, d_ow, d_ob,
              d_ident, d_lng, d_lnb, d_out)
    nc.compile()
    return nc


def _layernorm(nc, pool, x, out, g_bc, b_bc, eps_sb, pfx):
    """LN over last dim (24) of x [128, 8, 24] -> out [128, 8, 24]."""
    sums = pool.tile([128, 8], F32, name=f"{pfx}_sums", tag="ln_sums")
    nc.vector.tensor_reduce(sums, x, axis=mybir.AxisListType.X, op=OP.add)
    mean = pool.tile([128, 8], F32, name=f"{pfx}_mean", tag="ln_mean")
    nc.vector.tensor_scalar_mul(mean, sums, 1.0 / E)
    meanb = _ap(mean.tensor, mean.offset, [mean.ap[0], list(mean.ap[1]), [0, E]])
    cent = pool.tile([128, 8, E], F32, name=f"{pfx}_cent", tag="ln_cent")
    nc.vector.tensor_tensor(cent, x, meanb, OP.subtract)
    sq = pool.tile([128, 8, E], F32, name=f"{pfx}_sq", tag="ln_sq")
    nc.vector.tensor_tensor(sq, cent, cent, OP.mult)
    sqs = pool.tile([128, 8], F32, name=f"{pfx}_sqs", tag="ln_sqs")
    nc.vector.tensor_reduce(sqs, sq, axis=mybir.AxisListType.X, op=OP.add)
    var = pool.tile([128, 8], F32, name=f"{pfx}_var", tag="ln_var")
    nc.vector.tensor_scalar_mul(var, sqs, 1.0 / E)
    lnv = pool.tile([128, 8], F32, name=f"{pfx}_lnv", tag="ln_lnv")
    nc.scalar.activation(lnv, var, AF.Ln, bias=eps_sb, scale=1.0)
    rstd = pool.tile([128, 8], F32, name=f"{pfx}_rstd", tag="ln_rstd")
    nc.scalar.activation(rstd, lnv, AF.Exp, scale=-0.5)
    rstdb = _ap(rstd.tensor, rstd.offset, [rstd.ap[0], list(rstd.ap[1]), [0, E]])
    nrm = pool.tile([128, 8, E], F32, name=f"{pfx}_nrm", tag="ln_nrm")
    nc.vector.tensor_tensor(nrm, cent, rstdb, OP.mult)
    # apply g (broadcast over partitions and lt): g_bc is [128, 24]
    gv = _ap(g_bc.tensor, g_bc.offset, [g_bc.ap[0], [0, 8], list(g_bc.ap[1])])
    bv = _ap(b_bc.tensor, b_bc.offset, [b_bc.ap[0], [0, 8], list(b_bc.ap[1])])
    nc.vector.tensor_tensor(nrm, nrm, gv, OP.mult)
    nc.vector.tensor_tensor(out, nrm, bv, OP.add)


def _emit(tc, nc, erf_func, d_xpad, d_cw, d_cb, d_cberf, d_peT, d_wq, d_wk,
          d_wv, d_relU, d_w1, d_b1c, d_w2, d_b2, d_ow, d_ob, d_ident,
          d_lng, d_lnb, d_out):
    from contextlib import ExitStack
    ctx = ExitStack()
    with ctx:
        singles = ctx.enter_context(tc.tile_pool(name="singles", bufs=1))
        texp_pool = ctx.enter_context(tc.tile_pool(name="texp", bufs=3))
        aosb_pool = ctx.enter_context(tc.tile_pool(name="aosb", bufs=2))
        scratch = ctx.enter_context(tc.tile_pool(name="scratch", bufs=1))

        # ---- load params ----
        cw = singles.tile([KW, E], F32, name="cw_sb")
        nc.sync.dma_start(out=cw, in_=d_cw.ap())
        cb = singles.tile([E, 1], F32, name="cb_sb")
        nc.sync.dma_start(out=cb, in_=d_cb.ap())
        cberf = singles.tile([E, 1], F32, name="cberf_sb")
        nc.sync.dma_start(out=cberf, in_=d_cberf.ap())
        peT = singles.tile([E, L], F32, name="peT_sb")
        nc.sync.dma_start(out=peT, in_=d_peT.ap())
        wq = singles.tile([E, E], F32, name="wq_sb")
        nc.sync.dma_start(out=wq, in_=d_wq.ap())
        wk = singles.tile([E, E], F32, name="wk_sb")
        nc.sync.dma_start(out=wk, in_=d_wk.ap())
        wv = singles.tile([E, E], F32, name="wv_sb")
        nc.sync.dma_start(out=wv, in_=d_wv.ap())
        w1 = singles.tile([E, DFF], F32, name="w1_sb")
        nc.sync.dma_start(out=w1, in_=d_w1.ap())
        b1c = singles.tile([128, 2], F32, name="b1c_sb")
        nc.sync.dma_start(out=b1c, in_=d_b1c.ap())
        w2 = singles.tile([128, 2, E], F32, name="w2_sb")
        nc.sync.dma_start(out=w2, in_=d_w2.ap())
        b2 = singles.tile([E, 1], F32, name="b2_sb")
        nc.sync.dma_start(out=b2, in_=d_b2.ap())
        ow = singles.tile([E, NCls], F32, name="ow_sb")
        nc.sync.dma_start(out=ow, in_=d_ow.ap())
        ob = singles.tile([NCls, 1], F32, name="ob_sb")
        nc.sync.dma_start(out=ob, in_=d_ob.ap())
        ident = singles.tile([128, 128], F32, name="ident_sb")
        nc.sync.dma_start(out=ident, in_=d_ident.ap())
        # eRPE Toeplitz block weights, host-expanded: [128, H, 15*128] bf16
        u_all = singles.tile([128, H, 15 * 128], BF16, name="u_all")
        nc.sync.dma_start(out=u_all, in_=d_relU.ap())
        # LN g/b broadcast tiles [128, 24] each
        lng_bc = singles.tile([128, 3, E], F32, name="lng_bc")
        nc.sync.dma_start(out=lng_bc,
                          in_=_ap(d_lng, 0, [[0, 128], [E, 3], [1, E]]))
        lnb_bc = singles.tile([128, 3, E], F32, name="lnb_bc")
        nc.sync.dma_start(out=lnb_bc,
                          in_=_ap(d_lnb, 0, [[0, 128], [E, 3], [1, E]]))
        eps_sb = singles.tile([128, 1], F32, name="eps_sb")
        nc.vector.memset(eps_sb, EPS)
        # dependency-free dummy activations: preload ACT table sets so real
        # activation instructions don't carry the table-load waits
        dummy_act = singles.tile([1, 1], F32, name="dummy_act")
        nc.vector.memset(dummy_act, 0.5)
        nc.scalar.activation(dummy_act, dummy_act, AF.Exp, scale=1.0)
        nc.scalar.activation(dummy_act, dummy_act, AF.Ln, scale=1.0)
        nc.scalar.activation(dummy_act, dummy_act, erf_func, scale=1.0)
        ones128 = singles.tile([128, 1], F32, name="ones128")
        nc.vector.memset(ones128, 1.0)
        ones11 = singles.tile([1, 1], F32, name="ones11")
        nc.vector.memset(ones11, 1.0)
        z1 = singles.tile([1, 128], F32, name="z1_sb")
        nc.vector.memset(z1, 0.0)
        z2 = singles.tile([1, 192], F32, name="z2_sb")
        nc.vector.memset(z2, 0.0)

        # big single tiles
        V_sb = singles.tile([128, 8, 8, 4], BF16, name="V_sb")
        q_all = singles.tile([HD, H, L], BF16, name="q_all")
        k_all = singles.tile([HD, H, L], BF16, name="k_all")
        aoT_stack = singles.tile([32, L], F32, name="aoT_stack")
        xsrcT = singles.tile([E, L], F32, name="xsrcT")
        xposT = singles.tile([E, L], F32, name="xposT")

        # ============ phase 1: conv embed + BN + GELU + tAPE ============
        with tc.tile_pool(name="ph1ps", bufs=1, space="PSUM") as ph1ps, \
             tc.tile_pool(name="ph1sb", bufs=1) as ph1sb:
            xcol = ph1sb.tile([KW, L], F32, name="xcol")
            nc.sync.dma_start(out=xcol, in_=_ap(d_xpad, 0, [[1, KW], [1, L]]))
            conv_ps = ph1ps.tile([E, L], F32, name="conv_ps")
            for hh in range(2):
                nc.tensor.matmul(conv_ps[:, hh * 512:(hh + 1) * 512],
                                 cw,
                                 xcol[:, hh * 512:(hh + 1) * 512],
                                 start=True, stop=True)
            # exact GELU via erf: gelu(y) = y * (0.5 + 0.5*erf(y/sqrt(2)))
            e_t = ph1sb.tile([E, L], F32, name="e_t")
            nc.scalar.activation(e_t, conv_ps, erf_func, bias=cberf,
                                 scale=INV_SQRT2)
            y_t = ph1sb.tile([E, L], F32, name="y_t")
            nc.scalar.activation(y_t, conv_ps, AF.Identity, bias=cb, scale=1.0)
            t05 = ph1sb.tile([E, L], F32, name="t05")
            nc.vector.tensor_scalar(t05, e_t, 0.5, 0.5, OP.mult, OP.add)
            nc.vector.tensor_tensor(xsrcT, y_t, t05, OP.mult)
            nc.vector.tensor_tensor(xposT, xsrcT, peT, OP.add)

            # ---- Q^T, K^T projections + per-head repack ----
            for (w_, dst, nm) in ((wq, q_all, "q"), (wk, k_all, "k")):
                prj = ph1ps.tile([E, L], F32, name=f"prj_{nm}", tag="prj")
                for hh in range(2):
                    nc.tensor.matmul(prj[:, hh * 512:(hh + 1) * 512],
                                     w_,
                                     xposT[:, hh * 512:(hh + 1) * 512],
                                     start=True, stop=True)
                stg = ph1sb.tile([E, L], BF16, name=f"stg_{nm}", tag="stg")
                nc.vector.tensor_copy(stg, prj)
                for h in range(H):
                    nc.sync.dma_start(out=dst[:, h, :],
                                      in_=stg[3 * h:3 * h + 3, :])

            # ---- V in [L, head, dim|1] layout ----
            nc.vector.memset(V_sb, 1.0)
            for jt in range(8):
                vps = ph1ps.tile([128, E], F32, name=f"vps{jt}", tag="vps")
                nc.tensor.matmul(vps,
                                 xposT[:, jt * 128:(jt + 1) * 128],
                                 wv, start=True, stop=True)
                vview = _ap(vps.tensor, vps.offset, [vps.ap[0], [3, 8], [1, 3]])
                dst = _ap(V_sb.tensor, V_sb.offset + jt * 32,
                          [V_sb.ap[0], [4, 8], [1, 3]])
                nc.vector.tensor_copy(dst, vview)

        # ============ phase 2: attention ============
        with tc.tile_pool(name="biasps", bufs=1, space="PSUM") as biasps:
            bias_ps = biasps.tile([128, H, 8, HD], F32, name="bias_ps")
            flat = bias_ps.rearrange("p a b c -> p (a b c)")
            nc.tensor.matmul(flat, z1, z2, start=True, stop=False,
                             skip_group_check=True)

            with tc.tile_pool(name="sps", bufs=2, space="PSUM") as sps, \
                 tc.tile_pool(name="aops", bufs=1, space="PSUM") as aops:
                for h in range(H):
                    u_t = u_all[:, h, :]
                    ao_ps = aops.tile([4, L], F32, name=f"ao{h}", tag="ao")
                    for jt in range(8):
                        s_ps = sps.tile([128, L], F32, name=f"s{h}_{jt}",
                                        tag="s")
                        lw = k_all[:, h, jt * 128:(jt + 1) * 128]
                        for hh in range(2):
                            nc.tensor.matmul(
                                s_ps[:, hh * 512:(hh + 1) * 512], lw,
                                q_all[:, h, hh * 512:(hh + 1) * 512],
                                start=True, stop=True)
                        texp = texp_pool.tile([128, L], BF16,
                                              name=f"texp{h}_{jt}", tag="texp")
                        nc.scalar.activation(texp, s_ps, AF.Exp, scale=SCALE)
                        v1 = V_sb[:, jt, h, :]
                        for hh in range(2):
                            nc.tensor.matmul(
                                ao_ps[:, hh * 512:(hh + 1) * 512],
                                v1,
                                texp[:, hh * 512:(hh + 1) * 512],
                                start=(jt == 0), stop=(jt == 7))
                    # eRPE Toeplitz bias: 15 diagonal blocks
                    for d in range(-7, 8):
                        jt0 = max(0, -d)
                        n = 8 - abs(d)
                        it0 = max(0, d)
                        nc.tensor.matmul(
                            bias_ps[:, h, it0:it0 + n, :],
                            u_t[:, (d + 7) * 128:(d + 8) * 128],
                            V_sb[:, jt0:jt0 + n, h, 0:3],
                            start=False, stop=False, skip_group_check=True)
                    ao_sb = aosb_pool.tile([4, L], F32, name=f"aosb{h}",
                                           tag="aosb")
                    nc.vector.tensor_copy(ao_sb, ao_ps)
                    nc.sync.dma_start(out=aoT_stack[4 * h:4 * h + 4, :],
                                      in_=ao_sb)
                nc.tensor.matmul(flat, z1, z2, start=False, stop=True,
                                 skip_group_check=True)

            # ======== phase 3: transpose ao + z assembly ========
            z_sb = singles.tile([128, 8, E], F32, name="z_sb")
            with tc.tile_pool(name="trps", bufs=2, space="PSUM") as trps, \
                 tc.tile_pool(name="trsb", bufs=2) as trsb:
                for lt in range(8):
                    tr_ps = trps.tile([128, 32], F32, name=f"tr{lt}", tag="tr")
                    nc.tensor.transpose(tr_ps,
                                        aoT_stack[:, lt * 128:(lt + 1) * 128],
                                        ident[:32, :32])
                    tr_sb = trsb.tile([128, 8, 4], F32, name=f"trsb{lt}",
                                      tag="trs")
                    nc.vector.tensor_copy(tr_sb, tr_ps)
                    # ao = A * (1/d) + B  (d = denom col 3; B = bias_ps slice)
                    rec = trsb.tile([128, 8], F32, name=f"rec{lt}", tag="rec")
                    nc.vector.reciprocal(rec, tr_sb[:, :, 3])
                    recb = _ap(rec.tensor, rec.offset,
                               [rec.ap[0], list(rec.ap[1]), [0, 3]])
                    an = trsb.tile([128, 8, 3], F32, name=f"an{lt}", tag="an")
                    nc.vector.tensor_tensor(an, tr_sb[:, :, 0:3], recb,
                                            OP.mult)
                    nc.vector.tensor_tensor(z_sb[:, lt, :].rearrange(
                        "p (a b) -> p a b", a=8), an, bias_ps[:, :, lt, :],
                        OP.add)

        # ======== phase 4: LNs + FFN + pool + head ========
        y1 = singles.tile([128, 8, E], F32, name="y1_sb")
        att_L = singles.tile([128, 8, E], F32, name="attL_sb")
        y2 = singles.tile([128, 8, E], F32, name="y2_sb")
        out_L = singles.tile([128, 8, E], F32, name="outL_sb")
        zln = singles.tile([128, 8, E], F32, name="zln_sb")
        attT = singles.tile([E, L], F32, name="attT_sb")
        ffh0 = singles.tile([128, L], F32, name="ffh0_sb")
        ffh1 = singles.tile([128, L], F32, name="ffh1_sb")
        ffT = singles.tile([E, L], F32, name="ffT_sb")

        _layernorm(nc, scratch, z_sb, zln, lng_bc[:, 0, :], lnb_bc[:, 0, :],
                   eps_sb, "aln")
        with tc.tile_pool(name="xsps", bufs=2, space="PSUM") as xsps:
            for lt in range(8):
                xs_ps = xsps.tile([128, E], F32, name=f"xs{lt}", tag="xs")
                nc.tensor.transpose(xs_ps, xsrcT[:, lt * 128:(lt + 1) * 128],
                                    ident[:E, :E])
                nc.vector.tensor_tensor(y1[:, lt, :], zln[:, lt, :], xs_ps,
                                        OP.add)
        _layernorm(nc, scratch, y1, att_L, lng_bc[:, 1, :], lnb_bc[:, 1, :],
                   eps_sb, "ln1")

        with tc.tile_pool(name="atps", bufs=1, space="PSUM") as atps:
            attT_ps = atps.tile([E, L], F32, name="attT_ps")
            for lt in range(8):
                nc.tensor.transpose(attT_ps[:, lt * 128:(lt + 1) * 128],
                                    att_L[:, lt, :], ident)
            nc.vector.tensor_copy(attT, attT_ps)

        with tc.tile_pool(name="ffps", bufs=2, space="PSUM") as ffps:
            for p2, ffh in ((0, ffh0), (1, ffh1)):
                ffh_ps = ffps.tile([128, L], F32, name=f"ffh{p2}", tag="ffh")
                for hh in range(2):
                    nc.tensor.matmul(ffh_ps[:, hh * 512:(hh + 1) * 512],
                                     w1[:, p2 * 128:(p2 + 1) * 128],
                                     attT[:, hh * 512:(hh + 1) * 512],
                                     start=True, stop=True)
                nc.scalar.activation(ffh, ffh_ps, AF.Relu,
                                     bias=b1c[:, p2:p2 + 1], scale=1.0)

        with tc.tile_pool(name="f2ps", bufs=1, space="PSUM") as f2ps:
            ffT_ps = f2ps.tile([E, L], F32, name="ffT_ps")
            for hh in range(2):
                for p2, ffh in ((0, ffh0), (1, ffh1)):
                    nc.tensor.matmul(
                        ffT_ps[:, hh * 512:(hh + 1) * 512],
                        w2[:, p2, :],
                        ffh[:, hh * 512:(hh + 1) * 512],
                        start=(p2 == 0), stop=(p2 == 1))
            nc.scalar.activation(ffT, ffT_ps, AF.Identity, bias=b2, scale=1.0)

        with tc.tile_pool(name="fmps", bufs=2, space="PSUM") as fmps:
            for lt in range(8):
                ff_ps = fmps.tile([128, E], F32, name=f"ffm{lt}", tag="ffm")
                nc.tensor.transpose(ff_ps, ffT[:, lt * 128:(lt + 1) * 128],
                                    ident[:E, :E])
                nc.vector.tensor_tensor(y2[:, lt, :], att_L[:, lt, :], ff_ps,
                                        OP.add)
        _layernorm(nc, scratch, y2, out_L, lng_bc[:, 2, :], lnb_bc[:, 2, :],
                   eps_sb, "ln2")

        with tc.tile_pool(name="hdps", bufs=1, space="PSUM") as hdps, \
             tc.tile_pool(name="hdsb", bufs=1) as hdsb:
            pooled_ps = hdps.tile([1, E], F32, name="pooled_ps")
            for lt in range(8):
                nc.tensor.matmul(pooled_ps, ones128, out_L[:, lt, :],
                                 start=(lt == 0), stop=(lt == 7))
            pooled_sb = hdsb.tile([1, E], F32, name="pooled_sb")
            nc.vector.tensor_copy(pooled_sb, pooled_ps)
            pooledT_ps = hdps.tile([E, 1], F32, name="pooledT_ps")
            nc.tensor.matmul(pooledT_ps, pooled_sb, ones11, start=True,
                             stop=True)
            pooledT_sb = hdsb.tile([E, 1], F32, name="pooledT_sb")
            nc.vector.tensor_copy(pooledT_sb, pooledT_ps)
            logits_ps = hdps.tile([NCls, 1], F32, name="logits_ps")
            nc.tensor.matmul(logits_ps, ow, pooledT_sb, start=True, stop=True)
            logits_sb = hdsb.tile([NCls, 1], F32, name="logits_sb")
            nc.scalar.activation(logits_sb, logits_ps, AF.Identity, bias=ob,
                                 scale=1.0 / L)
            nc.sync.dma_start(out=d_out.ap(), in_=logits_sb)


def host_prep(inputs, erf=None):
    """Host-side parameter prep (tiny, O(E*K)). Returns (shared, per_core)."""
    f32 = np.float32
    a = (inputs["bn_gamma"] / np.sqrt(inputs["bn_var"] + EPS)).astype(f32)
    cw = (inputs["conv_w"][:, 0, :].T * a[None, :]).astype(f32)  # [K, E]
    cb = ((inputs["conv_b"] - inputs["bn_mean"]) * a
          + inputs["bn_beta"]).astype(f32).reshape(E, 1)
    # tAPE positional encoding
    pos = np.arange(L, dtype=f32)[:, None]
    div = np.exp(np.arange(0, E, 2, dtype=f32) * (-math.log(10000.0) / E))
    ang = pos * div * (float(E) / float(L))
    pe = np.zeros((L, E), f32)
    pe[:, 0::2] = np.sin(ang)
    pe[:, 1::2] = np.cos(ang)
    b1 = inputs["ff_b1"].astype(f32)
    b1c = np.stack([b1[:128], b1[128:]], axis=1)  # [128, 2]
    shared = {
        "cw": cw,
        "cb": cb,
        "cberf": (cb * INV_SQRT2).astype(f32),
        "peT": pe.T.copy(),
        "wq": inputs["wq"].astype(f32),
        "wk": inputs["wk"].astype(f32),
        "wv": inputs["wv"].astype(f32),
        # eRPE Toeplitz blocks, expanded: U[j', h, m] = table[127 - j' + m, h]
        "relU": np.ascontiguousarray(
            inputs["rel_bias_table"].astype(f32)[
                127 - np.arange(128)[:, None] + np.arange(15 * 128)[None, :]
            ].transpose(0, 2, 1)).astype(mybir.dt.np(BF16)),
        "w1": inputs["ff_w1"].astype(f32),
        "b1c": b1c.copy(),
        "w2": np.ascontiguousarray(
            inputs["ff_w2"].astype(f32).reshape(2, 128, E).transpose(1, 0, 2)),
        "b2": inputs["ff_b2"].astype(f32).reshape(E, 1),
        "ow": inputs["out_w"].astype(f32),
        "ob": inputs["out_b"].astype(f32).reshape(NCls, 1),
        "ident": np.eye(128, dtype=f32),
        "lng": np.stack([inputs["attn_ln_g"], inputs["ln1_g"],
                         inputs["ln2_g"]]).astype(f32),
        "lnb": np.stack([inputs["attn_ln_b"], inputs["ln1_b"],
                         inputs["ln2_b"]]).astype(f32),
    }
    x = inputs["x"].astype(f32)  # (B, 1, L)
    per_core = []
    for b in range(B):
        xpad = np.zeros((L + KW - 1,), f32)
        xpad[3:3 + L] = x[b, 0]
        per_core.append({"xpad": xpad, **shared})
    return per_core


_NC_CACHE = {}


def kernel(**inputs) -> np.ndarray:
    from concourse.bass_utils import run_bass_kernel_spmd
    if "nc" not in _NC_CACHE:
        _NC_CACHE["nc"] = build_nc()
    nc = _NC_CACHE["nc"]
    in_maps = host_prep(inputs)
    res = run_bass_kernel_spmd(nc, in_maps, core_ids=list(range(NCORES)))
    out = np.stack([res.results[b]["out"].reshape(NCls) for b in range(B)])
    return out.astype(np.float32)


if __name__ == "__main__":
    import reference
    ins = {k: np.asarray(v) for k, v in reference.setup_inputs().items()}
    got = kernel(**ins)
    exp = np.asarray(reference.reference(**reference.setup_inputs()))
    err = np.abs(got - exp).max() / np.abs(exp).max()
    print("Relative error:", err)



# revision 11
# speedup vs baseline: 10.7145x; 1.1987x over previous
"""Bass/Tile TRN2 kernel for nn_ConvTran_618475290811.

ConvTran tiny transformer: conv embed + BN + GELU + tAPE + eRPE attention
(bias added AFTER softmax) + FFN + mean-pool + classifier head.
B=8 batch elements, data-parallel one per NeuronCore (8 cores).

Key tricks:
 - attention computed in transposed (S^T = [keys, queries]) layout; softmax
   denominator produced for free via a ones-column appended to V.
 - no division for softmax: LayerNorm scale-invariance lets us feed
   z = exp@v + denom * (R@v) into the to_out LayerNorm.
 - eRPE Toeplitz bias R@v via 15 diagonal-block stationary weights per head,
   host-expanded into a [128, H, 15*128] bf16 tensor (single contiguous DMA).
 - 4-head PE packing: S matmuls (K=3) row-tiled at tile_position=(32j, 0),
   AO matmuls (M=4) col-tiled at tile_position=(0, 32j) - 4 run concurrently.
 - one EXP per (group, key-tile, query-half) over N=2048 (4 PSUM banks).
 - LayerNorm rstd via Sqrt + DVE reciprocal (single ACT table set in tail).
"""
import math
import numpy as np

import concourse.bass as bass
import concourse.bacc as bacc
import concourse.tile as tile
from concourse import mybir

B, L, E, H, NCls, DFF, KW = 8, 1024, 24, 8, 10, 256, 8
HD = E // H  # 3
NCORES = 8
F32 = mybir.dt.float32
BF16 = mybir.dt.bfloat16
AF = mybir.ActivationFunctionType
OP = mybir.AluOpType
SCALE = float(E) ** -0.5
INV_SQRT2 = 0.7071067811865476
EPS = 1e-5


def _ap(t, off, pattern):
    return bass.AP(t, off, pattern)


def build_nc(erf_func=AF.Erf):
    nc = bacc.Bacc("TRN2", target_bir_lowering=False, debug=False)

    # ---- DRAM I/O ----
    d_xpad = nc.dram_tensor("xpad", [L + KW - 1], F32, kind="ExternalInput")
    d_cw = nc.dram_tensor("cw", [KW, E], F32, kind="ExternalInput")
    d_cb = nc.dram_tensor("cb", [E, 1], F32, kind="ExternalInput")
    d_cberf = nc.dram_tensor("cberf", [E, 1], F32, kind="ExternalInput")
    d_peT = nc.dram_tensor("peT", [E, L], F32, kind="ExternalInput")
    d_wq = nc.dram_tensor("wq", [E, E], F32, kind="ExternalInput")
    d_wk = nc.dram_tensor("wk", [E, E], F32, kind="ExternalInput")
    d_wv = nc.dram_tensor("wv", [E, E], F32, kind="ExternalInput")
    d_relU = nc.dram_tensor("relU", [128, H, 15 * 128], BF16,
                            kind="ExternalInput")
    d_w1 = nc.dram_tensor("w1", [E, DFF], F32, kind="ExternalInput")
    d_b1c = nc.dram_tensor("b1c", [128, 2], F32, kind="ExternalInput")
    d_w2 = nc.dram_tensor("w2", [128, 2, E], F32, kind="ExternalInput")
    d_b2 = nc.dram_tensor("b2", [E, 1], F32, kind="ExternalInput")
    d_ow = nc.dram_tensor("ow", [E, NCls], F32, kind="ExternalInput")
    d_ob = nc.dram_tensor("ob", [NCls, 1], F32, kind="ExternalInput")
    d_ident = nc.dram_tensor("ident", [128, 128], F32, kind="ExternalInput")
    # 6 LayerNorm gain/bias rows: attn_ln, ln1, ln2
    d_lng = nc.dram_tensor("lng", [3, E], F32, kind="ExternalInput")
    d_lnb = nc.dram_tensor("lnb", [3, E], F32, kind="ExternalInput")
    d_out = nc.dram_tensor("out", [NCls, 1], F32, kind="ExternalOutput")

    with tile.TileContext(nc) as tc:
        _emit(tc, nc, erf_func, d_xpad, d_cw, d_cb, d_cberf, d_peT, d_wq,
              d_wk, d_wv, d_relU, d_w1, d_b1c, d_w2, d_b2, d_ow, d_ob,
              d_ident, d_lng, d_lnb, d_out)
    nc.compile()
    return nc


def _layernorm(nc, pool, x, out, g_bc, b_bc, eps_sb, pfx):
    """LN over last dim (24) of x [128, 8, 24] -> out [128, 8, 24].

    rstd via ACT Sqrt + DVE reciprocal (keeps everything in one table set).
    """
    sums = pool.tile([128, 8], F32, name=f"{pfx}_sums", tag="ln_sums")
    nc.vector.tensor_reduce(sums, x, axis=mybir.AxisListType.X, op=OP.add)
    sumsb = _ap(sums.tensor, sums.offset,
                [sums.ap[0], list(sums.ap[1]), [0, E]])
    cent = pool.tile([128, 8, E], F32, name=f"{pfx}_cent", tag="ln_cent")
    # cent = x - sums/E
    nc.vector.scalar_tensor_tensor(cent, sumsb, -1.0 / E, x, OP.mult, OP.add)
    sq = pool.tile([128, 8, E], F32, name=f"{pfx}_sq", tag="ln_sq")
    nc.vector.tensor_tensor(sq, cent, cent, OP.mult)
    sqs = pool.tile([128, 8], F32, name=f"{pfx}_sqs", tag="ln_sqs")
    nc.vector.tensor_reduce(sqs, sq, axis=mybir.AxisListType.X, op=OP.add)
    std = pool.tile([128, 8], F32, name=f"{pfx}_std", tag="ln_std")
    nc.scalar.activation(std, sqs, AF.Sqrt, bias=eps_sb, scale=1.0 / E)
    rstd = pool.tile([128, 8], F32, name=f"{pfx}_rstd", tag="ln_rstd")
    nc.vector.reciprocal(rstd, std)
    rstdb = _ap(rstd.tensor, rstd.offset,
                [rstd.ap[0], list(rstd.ap[1]), [0, E]])
    nrm = pool.tile([128, 8, E], F32, name=f"{pfx}_nrm", tag="ln_nrm")
    nc.vector.tensor_tensor(nrm, cent, rstdb, OP.mult)
    # apply g, b (broadcast over partitions and lt): g_bc is [128, 24]
    gv = _ap(g_bc.tensor, g_bc.offset, [g_bc.ap[0], [0, 8], list(g_bc.ap[1])])
    bv = _ap(b_bc.tensor, b_bc.offset, [b_bc.ap[0], [0, 8], list(b_bc.ap[1])])
    nc.vector.tensor_tensor(nrm, nrm, gv, OP.mult)
    nc.vector.tensor_tensor(out, nrm, bv, OP.add)


def _emit(tc, nc, erf_func, d_xpad, d_cw, d_cb, d_cberf, d_peT, d_wq, d_wk,
          d_wv, d_relU, d_w1, d_b1c, d_w2, d_b2, d_ow, d_ob, d_ident,
          d_lng, d_lnb, d_out):
    from contextlib import ExitStack
    ctx = ExitStack()
    with ctx:
        singles = ctx.enter_context(tc.tile_pool(name="singles", bufs=1))
        texp_pool = ctx.enter_context(tc.tile_pool(name="texp", bufs=2))
        scratch = ctx.enter_context(tc.tile_pool(name="scratch", bufs=1))

        # ---- phase-1-critical loads first (conv + projections) ----
        xcol = singles.tile([KW, L], F32, name="xcol")
        nc.sync.dma_start(out=xcol, in_=_ap(d_xpad, 0, [[1, KW], [1, L]]))
        cw = singles.tile([KW, E], F32, name="cw_sb")
        nc.sync.dma_start(out=cw, in_=d_cw.ap())
        cb = singles.tile([E, 1], F32, name="cb_sb")
        nc.sync.dma_start(out=cb, in_=d_cb.ap())
        cberf = singles.tile([E, 1], F32, name="cberf_sb")
        nc.sync.dma_start(out=cberf, in_=d_cberf.ap())
        peT = singles.tile([E, L], F32, name="peT_sb")
        nc.sync.dma_start(out=peT, in_=d_peT.ap())
        wq = singles.tile([E, E], F32, name="wq_sb")
        nc.sync.dma_start(out=wq, in_=d_wq.ap())
        wk = singles.tile([E, E], F32, name="wk_sb")
        nc.sync.dma_start(out=wk, in_=d_wk.ap())
        wv = singles.tile([E, E], F32, name="wv_sb")
        nc.sync.dma_start(out=wv, in_=d_wv.ap())
        # eRPE Toeplitz block weights, host-expanded: [128, H, 15*128] bf16
        u_all = singles.tile([128, H, 15 * 128], BF16, name="u_all")
        nc.sync.dma_start(out=u_all, in_=d_relU.ap())
        # ---- later-phase params ----
        ident = singles.tile([128, 128], F32, name="ident_sb")
        nc.sync.dma_start(out=ident, in_=d_ident.ap())
        w1 = singles.tile([E, DFF], F32, name="w1_sb")
        nc.sync.dma_start(out=w1, in_=d_w1.ap())
        b1c = singles.tile([128, 2], F32, name="b1c_sb")
        nc.sync.dma_start(out=b1c, in_=d_b1c.ap())
        w2 = singles.tile([128, 2, E], F32, name="w2_sb")
        nc.sync.dma_start(out=w2, in_=d_w2.ap())
        b2 = singles.tile([E, 1], F32, name="b2_sb")
        nc.sync.dma_start(out=b2, in_=d_b2.ap())
        ow = singles.tile([E, NCls], F32, name="ow_sb")
        nc.sync.dma_start(out=ow, in_=d_ow.ap())
        ob = singles.tile([NCls, 1], F32, name="ob_sb")
        nc.sync.dma_start(out=ob, in_=d_ob.ap())
        lng_bc = singles.tile([128, 3, E], F32, name="lng_bc")
        nc.sync.dma_start(out=lng_bc,
                          in_=_ap(d_lng, 0, [[0, 128], [E, 3], [1, E]]))
        lnb_bc = singles.tile([128, 3, E], F32, name="lnb_bc")
        nc.sync.dma_start(out=lnb_bc,
                          in_=_ap(d_lnb, 0, [[0, 128], [E, 3], [1, E]]))
        eps_sb = singles.tile([128, 1], F32, name="eps_sb")
        nc.vector.memset(eps_sb, EPS)
        # dummy activation: preload the erf table set before phase 1 uses it
        dummy_act = singles.tile([1, 1], F32, name="dummy_act")
        nc.vector.memset(dummy_act, 0.5)
        nc.scalar.activation(dummy_act, dummy_act, erf_func, scale=1.0)
        ones128 = singles.tile([128, 1], F32, name="ones128")
        nc.vector.memset(ones128, 1.0)
        ones11 = singles.tile([1, 1], F32, name="ones11")
        nc.vector.memset(ones11, 1.0)
        z1 = singles.tile([1, 128], F32, name="z1_sb")
        nc.vector.memset(z1, 0.0)
        z2 = singles.tile([1, 192], F32, name="z2_sb")
        nc.vector.memset(z2, 0.0)

        # big single tiles
        # V in [key-in-tile, jt, head, dim|1] layout (col 3 = ones for denom)
        V_sb = singles.tile([128, 8, 8, 4], BF16, name="V_sb")
        # Q/K in 4-head-strip layout: head 4g+j at partitions 32j..32j+2
        q4 = singles.tile([128, 2, L], BF16, name="q4")
        k4 = singles.tile([128, 2, L], BF16, name="k4")
        aoT_stack = singles.tile([32, L], F32, name="aoT_stack")
        xsrcT = singles.tile([E, L], F32, name="xsrcT")
        xposT = singles.tile([E, L], F32, name="xposT")

        # ============ phase 1: conv embed + BN + GELU + tAPE ============
        with tc.tile_pool(name="ph1ps", bufs=1, space="PSUM") as ph1ps, \
             tc.tile_pool(name="prjps", bufs=2, space="PSUM") as prjps, \
             tc.tile_pool(name="ph1sb", bufs=1) as ph1sb:
            conv_ps = ph1ps.tile([E, L], F32, name="conv_ps")
            for hh in range(2):
                nc.tensor.matmul(conv_ps[:, hh * 512:(hh + 1) * 512],
                                 cw,
                                 xcol[:, hh * 512:(hh + 1) * 512],
                                 start=True, stop=True)
            # exact GELU via erf: gelu(y) = y * (0.5 + 0.5*erf(y/sqrt(2)))
            e_t = ph1sb.tile([E, L], F32, name="e_t")
            nc.scalar.activation(e_t, conv_ps, erf_func, bias=cberf,
                                 scale=INV_SQRT2)
            y_t = ph1sb.tile([E, L], F32, name="y_t")
            nc.scalar.activation(y_t, conv_ps, AF.Identity, bias=cb, scale=1.0)
            # preload exp table set while projections run on PE
            nc.scalar.activation(dummy_act, dummy_act, AF.Exp, scale=1.0)
            t05 = ph1sb.tile([E, L], F32, name="t05")
            nc.vector.tensor_scalar(t05, e_t, 0.5, 0.5, OP.mult, OP.add)
            nc.vector.tensor_tensor(xsrcT, y_t, t05, OP.mult)
            nc.vector.tensor_tensor(xposT, xsrcT, peT, OP.add)

            # ---- Q^T, K^T projections + per-head repack to 32-strips ----
            # head 4g+j lands at partitions 32j..32j+2 of q4/k4[:, g, :]
            for (w_, dst, nm) in ((wq, q4, "q"), (wk, k4, "k")):
                prj = prjps.tile([E, L], F32, name=f"prj_{nm}", tag="prj")
                for hh in range(2):
                    nc.tensor.matmul(prj[:, hh * 512:(hh + 1) * 512],
                                     w_,
                                     xposT[:, hh * 512:(hh + 1) * 512],
                                     start=True, stop=True)
                stg = ph1sb.tile([E, L], BF16, name=f"stg_{nm}", tag="stg")
                nc.vector.tensor_copy(stg, prj)
                for h in range(H):
                    g, j = h // 4, h % 4
                    nc.sync.dma_start(out=dst[32 * j:32 * j + 3, g, :],
                                      in_=stg[3 * h:3 * h + 3, :])

            # ---- V in [key, jt, head, dim|1] layout ----
            nc.vector.memset(V_sb, 1.0)
            for jt in range(8):
                vps = ph1ps.tile([128, E], F32, name=f"vps{jt}", tag="vps")
                nc.tensor.matmul(vps,
                                 xposT[:, jt * 128:(jt + 1) * 128],
                                 wv, start=True, stop=True)
                vview = _ap(vps.tensor, vps.offset, [vps.ap[0], [3, 8], [1, 3]])
                dst = _ap(V_sb.tensor, V_sb.offset + jt * 32,
                          [V_sb.ap[0], [4, 8], [1, 3]])
                nc.vector.tensor_copy(dst, vview)

        # ============ phase 2: attention (4-head packed) ============
        with tc.tile_pool(name="biasps", bufs=1, space="PSUM") as biasps:
            bias_ps = biasps.tile([128, H, 8, HD], F32, name="bias_ps")
            flat = bias_ps.rearrange("p a b c -> p (a b c)")
            nc.tensor.matmul(flat, z1, z2, start=True, stop=False,
                             skip_group_check=True)

            ao_sb = [None, None]
            with tc.tile_pool(name="sps", bufs=1, space="PSUM") as sps, \
                 tc.tile_pool(name="aops", bufs=1, space="PSUM") as aops, \
                 tc.tile_pool(name="aosb", bufs=2) as aosb_pool:
                for g in range(2):
                    ao_ps = aops.tile([128, L], F32, name=f"ao{g}", tag="ao")
                    # 60 bias matmuls per group, interleaved into the 16
                    # (jt, hh) steps to fill the PE idle under EXP
                    bias_work = []
                    for j in range(4):
                        h = 4 * g + j
                        for d in range(-7, 8):
                            bias_work.append((h, d))
                    bi = 0
                    for jt in range(8):
                        for hh in range(2):
                            s4 = sps.tile([128, 4, 512], F32,
                                          name=f"s{g}_{jt}_{hh}", tag="s")
                            for j in range(4):
                                h = 4 * g + j
                                nc.tensor.matmul(
                                    s4[:, j, :],
                                    k4[32 * j:32 * j + 3, g,
                                       jt * 128:(jt + 1) * 128],
                                    q4[32 * j:32 * j + 3, g,
                                       hh * 512:(hh + 1) * 512],
                                    start=True, stop=True,
                                    tile_position=(32 * j, 0),
                                    skip_group_check=True)
                            texp = texp_pool.tile([128, 4, 512], BF16,
                                                  name=f"tx{g}{jt}{hh}",
                                                  tag="texp")
                            nc.scalar.activation(texp, s4, AF.Exp,
                                                 scale=SCALE)
                            for j in range(4):
                                h = 4 * g + j
                                nc.tensor.matmul(
                                    ao_ps[32 * j:32 * j + 4,
                                          hh * 512:(hh + 1) * 512],
                                    V_sb[:, jt, h, :],
                                    texp[:, j, :],
                                    start=(jt == 0), stop=(jt == 7),
                                    tile_position=(0, 32 * j),
                                    skip_group_check=True)
                            # ~4 bias matmuls per step (60 over 16 steps)
                            n_this = (60 * (2 * jt + hh + 1)) // 16 - bi
                            for _ in range(n_this):
                                h, d = bias_work[bi]
                                bi += 1
                                jt0 = max(0, -d)
                                n = 8 - abs(d)
                                it0 = max(0, d)
                                nc.tensor.matmul(
                                    bias_ps[:, h, it0:it0 + n, :],
                                    u_all[:, h, (d + 7) * 128:(d + 8) * 128],
                                    V_sb[:, jt0:jt0 + n, h, 0:3],
                                    start=False, stop=False,
                                    skip_group_check=True)
                    ao_sb[g] = aosb_pool.tile([128, L], F32, name=f"aosb{g}",
                                              tag="aosb")
                    for j in range(4):
                        h = 4 * g + j
                        nc.vector.tensor_copy(
                            ao_sb[g][32 * j:32 * j + 4, :],
                            ao_ps[32 * j:32 * j + 4, :])
                        nc.sync.dma_start(
                            out=aoT_stack[4 * h:4 * h + 4, :],
                            in_=ao_sb[g][32 * j:32 * j + 4, :])
                nc.tensor.matmul(flat, z1, z2, start=False, stop=True,
                                 skip_group_check=True)
            # preload sqrt table set while phase-3 transposes run
            nc.scalar.activation(dummy_act, dummy_act, AF.Sqrt, scale=1.0)

            # ======== phase 3: transpose ao + z assembly ========
            z_sb = singles.tile([128, 8, E], F32, name="z_sb")
            with tc.tile_pool(name="trps", bufs=2, space="PSUM") as trps, \
                 tc.tile_pool(name="trsb", bufs=2) as trsb:
                for lt in range(8):
                    tr_ps = trps.tile([128, 32], F32, name=f"tr{lt}", tag="tr")
                    nc.tensor.transpose(tr_ps,
                                        aoT_stack[:, lt * 128:(lt + 1) * 128],
                                        ident[:32, :32])
                    tr_sb = trsb.tile([128, 8, 4], F32, name=f"trsb{lt}",
                                      tag="trs")
                    nc.vector.tensor_copy(tr_sb, tr_ps)
                    # ao = A * (1/d) + B  (d = denom col 3; B = bias_ps slice)
                    rec = trsb.tile([128, 8], F32, name=f"rec{lt}", tag="rec")
                    nc.vector.reciprocal(rec, tr_sb[:, :, 3])
                    recb = _ap(rec.tensor, rec.offset,
                               [rec.ap[0], list(rec.ap[1]), [0, 3]])
                    an = trsb.tile([128, 8, 3], F32, name=f"an{lt}", tag="an")
                    nc.vector.tensor_tensor(an, tr_sb[:, :, 0:3], recb,
                                            OP.mult)
                    nc.vector.tensor_tensor(z_sb[:, lt, :].rearrange(
                        "p (a b) -> p a b", a=8), an, bias_ps[:, :, lt, :],
                        OP.add)

        # ======== phase 4: LNs + FFN + pool + head ========
        y1 = singles.tile([128, 8, E], F32, name="y1_sb")
        att_L = singles.tile([128, 8, E], F32, name="attL_sb")
        y2 = singles.tile([128, 8, E], F32, name="y2_sb")
        out_L = singles.tile([128, 8, E], F32, name="outL_sb")
        zln = singles.tile([128, 8, E], F32, name="zln_sb")
        attT = singles.tile([E, L], F32, name="attT_sb")
        ffh0 = singles.tile([128, L], F32, name="ffh0_sb")
        ffh1 = singles.tile([128, L], F32, name="ffh1_sb")
        ffT = singles.tile([E, L], F32, name="ffT_sb")

        _layernorm(nc, scratch, z_sb, zln, lng_bc[:, 0, :], lnb_bc[:, 0, :],
                   eps_sb, "aln")
        with tc.tile_pool(name="xsps", bufs=2, space="PSUM") as xsps:
            for lt in range(8):
                xs_ps = xsps.tile([128, E], F32, name=f"xs{lt}", tag="xs")
                nc.tensor.transpose(xs_ps, xsrcT[:, lt * 128:(lt + 1) * 128],
                                    ident[:E, :E])
                nc.vector.tensor_tensor(y1[:, lt, :], zln[:, lt, :], xs_ps,
                                        OP.add)
        _layernorm(nc, scratch, y1, att_L, lng_bc[:, 1, :], lnb_bc[:, 1, :],
                   eps_sb, "ln1")

        with tc.tile_pool(name="atps", bufs=1, space="PSUM") as atps:
            attT_ps = atps.tile([E, L], F32, name="attT_ps")
            for lt in range(8):
                nc.tensor.transpose(attT_ps[:, lt * 128:(lt + 1) * 128],
                                    att_L[:, lt, :], ident)
            nc.vector.tensor_copy(attT, attT_ps)

        with tc.tile_pool(name="ffps", bufs=2, space="PSUM") as ffps:
            for p2, ffh in ((0, ffh0), (1, ffh1)):
                ffh_ps = ffps.tile([128, L], F32, name=f"ffh{p2}", tag="ffh")
                for hh in range(2):
                    nc.tensor.matmul(ffh_ps[:, hh * 512:(hh + 1) * 512],
                                     w1[:, p2 * 128:(p2 + 1) * 128],
                                     attT[:, hh * 512:(hh + 1) * 512],
                                     start=True, stop=True)
                nc.scalar.activation(ffh, ffh_ps, AF.Relu,
                                     bias=b1c[:, p2:p2 + 1], scale=1.0)

        with tc.tile_pool(name="f2ps", bufs=1, space="PSUM") as f2ps:
            ffT_ps = f2ps.tile([E, L], F32, name="ffT_ps")
            for hh in range(2):
                for p2, ffh in ((0, ffh0), (1, ffh1)):
                    nc.tensor.matmul(
                        ffT_ps[:, hh * 512:(hh + 1) * 512],
                        w2[:, p2, :],
                        ffh[:, hh * 512:(hh + 1) * 512],
                        start=(p2 == 0), stop=(p2 == 1))
            nc.scalar.activation(ffT, ffT_ps, AF.Identity, bias=b2, scale=1.0)

        with tc.tile_pool(name="fmps", bufs=2, space="PSUM") as fmps:
            for lt in range(8):
                ff_ps = fmps.tile([128, E], F32, name=f"ffm{lt}", tag="ffm")
                nc.tensor.transpose(ff_ps, ffT[:, lt * 128:(lt + 1) * 128],
                                    ident[:E, :E])
                nc.vector.tensor_tensor(y2[:, lt, :], att_L[:, lt, :], ff_ps,
                                        OP.add)
        _layernorm(nc, scratch, y2, out_L, lng_bc[:, 2, :], lnb_bc[:, 2, :],
                   eps_sb, "ln2")

        with tc.tile_pool(name="hdps", bufs=1, space="PSUM") as hdps, \
             tc.tile_pool(name="hdsb", bufs=1) as hdsb:
            pooled_ps = hdps.tile([1, E], F32, name="pooled_ps")
            for lt in range(8):
                nc.tensor.matmul(pooled_ps, ones128, out_L[:, lt, :],
                                 start=(lt == 0), stop=(lt == 7))
            pooled_sb = hdsb.tile([1, E], F32, name="pooled_sb")
            nc.vector.tensor_copy(pooled_sb, pooled_ps)
            pooledT_ps = hdps.tile([E, 1], F32, name="pooledT_ps")
            nc.tensor.matmul(pooledT_ps, pooled_sb, ones11, start=True,
                             stop=True)
            pooledT_sb = hdsb.tile([E, 1], F32, name="pooledT_sb")
            nc.vector.tensor_copy(pooledT_sb, pooledT_ps)
            logits_ps = hdps.tile([NCls, 1], F32, name="logits_ps")
            nc.tensor.matmul(logits_ps, ow, pooledT_sb, start=True, stop=True)
            logits_sb = hdsb.tile([NCls, 1], F32, name="logits_sb")
            nc.scalar.activation(logits_sb, logits_ps, AF.Identity, bias=ob,
                                 scale=1.0 / L)
            nc.sync.dma_start(out=d_out.ap(), in_=logits_sb)


def host_prep(inputs, erf=None):
    """Host-side parameter prep (tiny, O(E*K)). Returns (shared, per_core)."""
    f32 = np.float32
    a = (inputs["bn_gamma"] / np.sqrt(inputs["bn_var"] + EPS)).astype(f32)
    cw = (inputs["conv_w"][:, 0, :].T * a[None, :]).astype(f32)  # [K, E]
    cb = ((inputs["conv_b"] - inputs["bn_mean"]) * a
          + inputs["bn_beta"]).astype(f32).reshape(E, 1)
    # tAPE positional encoding
    pos = np.arange(L, dtype=f32)[:, None]
    div = np.exp(np.arange(0, E, 2, dtype=f32) * (-math.log(10000.0) / E))
    ang = pos * div * (float(E) / float(L))
    pe = np.zeros((L, E), f32)
    pe[:, 0::2] = np.sin(ang)
    pe[:, 1::2] = np.cos(ang)
    b1 = inputs["ff_b1"].astype(f32)
    b1c = np.stack([b1[:128], b1[128:]], axis=1)  # [128, 2]
    shared = {
        "cw": cw,
        "cb": cb,
        "cberf": (cb * INV_SQRT2).astype(f32),
        "peT": pe.T.copy(),
        "wq": inputs["wq"].astype(f32),
        "wk": inputs["wk"].astype(f32),
        "wv": inputs["wv"].astype(f32),
        # eRPE Toeplitz blocks, expanded: U[j', h, m] = table[127 - j' + m, h]
        "relU": np.ascontiguousarray(
            inputs["rel_bias_table"].astype(f32)[
                127 - np.arange(128)[:, None] + np.arange(15 * 128)[None, :]
            ].transpose(0, 2, 1)).astype(mybir.dt.np(BF16)),
        "w1": inputs["ff_w1"].astype(f32),
        "b1c": b1c.copy(),
        "w2": np.ascontiguousarray(
            inputs["ff_w2"].astype(f32).reshape(2, 128, E).transpose(1, 0, 2)),
        "b2": inputs["ff_b2"].astype(f32).reshape(E, 1),
        "ow": inputs["out_w"].astype(f32),
        "ob": inputs["out_b"].astype(f32).reshape(NCls, 1),
        "ident": np.eye(128, dtype=f32),
        "lng": np.stack([inputs["attn_ln_g"], inputs["ln1_g"],
                         inputs["ln2_g"]]).astype(f32),
        "lnb": np.stack([inputs["attn_ln_b"], inputs["ln1_b"],
                         inputs["ln2_b"]]).astype(f32),
    }
    x = inputs["x"].astype(f32)  # (B, 1, L)
    per_core = []
    for b in range(B):
        xpad = np.zeros((L + KW - 1,), f32)
        xpad[3:3 + L] = x[b, 0]
        per_core.append({"xpad": xpad, **shared})
    return per_core


_NC_CACHE = {}


def kernel(**inputs) -> np.ndarray:
    from concourse.bass_utils import run_bass_kernel_spmd
    if "nc" not in _NC_CACHE:
        _NC_CACHE["nc"] = build_nc()
    nc = _NC_CACHE["nc"]
    in_maps = host_prep(inputs)
    res = run_bass_kernel_spmd(nc, in_maps, core_ids=list(range(NCORES)))
    out = np.stack([res.results[b]["out"].reshape(NCls) for b in range(B)])
    return out.astype(np.float32)


if __name__ == "__main__":
    import reference
    ins = {k: np.asarray(v) for k, v in reference.setup_inputs().items()}
    got = kernel(**ins)
    exp = np.asarray(reference.reference(**reference.setup_inputs()))
    err = np.abs(got - exp).max() / np.abs(exp).max()
    print("Relative error:", err)


# revision 23
# speedup vs baseline: 12.9482x; 1.2085x over previous
"""Bass/Tile TRN2 kernel for nn_ConvTran_618475290811.

ConvTran tiny transformer: conv embed + BN + GELU + tAPE + eRPE attention
(bias added AFTER softmax) + FFN + mean-pool + classifier head.
B=8 batch elements, data-parallel one per NeuronCore (8 cores).

Key tricks:
 - attention computed in transposed (S^T = [keys, queries]) layout; softmax
   denominator produced for free via a ones-column appended to V.
 - no division for softmax: LayerNorm scale-invariance lets us feed
   z = exp@v + denom * (R@v) into the to_out LayerNorm.
 - eRPE Toeplitz bias R@v via 15 diagonal-block stationary weights per head,
   host-expanded into a [128, H, 15*128] bf16 tensor (single contiguous DMA).
 - 4-head PE packing: S matmuls (K=3) row-tiled at tile_position=(32j, 0),
   AO matmuls (M=4) col-tiled at tile_position=(0, 32j) - 4 run concurrently.
 - one EXP per (group, key-tile, query-half) over N=2048 (4 PSUM banks).
 - LayerNorm rstd via Sqrt + DVE reciprocal (single ACT table set in tail).
"""
import math
import numpy as np

import concourse.bass as bass
import concourse.bacc as bacc
import concourse.tile as tile
from concourse import mybir

B, L, E, H, NCls, DFF, KW = 8, 1024, 24, 8, 10, 256, 8
HD = E // H  # 3
NCORES = 8
F32 = mybir.dt.float32
BF16 = mybir.dt.bfloat16
AF = mybir.ActivationFunctionType
OP = mybir.AluOpType
SCALE = float(E) ** -0.5
INV_SQRT2 = 0.7071067811865476
EPS = 1e-5


def _ap(t, off, pattern):
    return bass.AP(t, off, pattern)


def build_nc(erf_func=AF.Erf):
    nc = bacc.Bacc("TRN2", target_bir_lowering=False, debug=False)

    # ---- DRAM I/O ----
    d_xpad = nc.dram_tensor("xpad", [L + KW - 1], F32, kind="ExternalInput")
    d_cw = nc.dram_tensor("cw", [KW, E], F32, kind="ExternalInput")
    d_cb = nc.dram_tensor("cb", [E, 1], F32, kind="ExternalInput")
    d_cberf = nc.dram_tensor("cberf", [E, 1], F32, kind="ExternalInput")
    d_peT = nc.dram_tensor("peT", [E, L], F32, kind="ExternalInput")
    d_wq = nc.dram_tensor("wq", [E, E], BF16, kind="ExternalInput")
    d_wk = nc.dram_tensor("wk", [E, E], BF16, kind="ExternalInput")
    d_wv = nc.dram_tensor("wv", [E, E], BF16, kind="ExternalInput")
    d_relU = nc.dram_tensor("relU", [128, H, 15 * 128], BF16,
                            kind="ExternalInput")
    d_w1 = nc.dram_tensor("w1", [E, DFF], BF16, kind="ExternalInput")
    d_b1c = nc.dram_tensor("b1c", [128, 2], F32, kind="ExternalInput")
    d_w2 = nc.dram_tensor("w2", [128, 2, E], BF16, kind="ExternalInput")
    d_b2 = nc.dram_tensor("b2", [E, 1], F32, kind="ExternalInput")
    d_ow = nc.dram_tensor("ow", [E, NCls], F32, kind="ExternalInput")
    d_ob = nc.dram_tensor("ob", [NCls, 1], F32, kind="ExternalInput")
    d_ident = nc.dram_tensor("ident", [128, 128], F32, kind="ExternalInput")
    # 6 LayerNorm gain/bias rows: attn_ln, ln1, ln2
    d_lng = nc.dram_tensor("lng", [3, E], F32, kind="ExternalInput")
    d_lnb = nc.dram_tensor("lnb", [3, E], F32, kind="ExternalInput")
    d_out = nc.dram_tensor("out", [NCls, 1], F32, kind="ExternalOutput")

    with tile.TileContext(nc) as tc:
        _emit(tc, nc, erf_func, d_xpad, d_cw, d_cb, d_cberf, d_peT, d_wq,
              d_wk, d_wv, d_relU, d_w1, d_b1c, d_w2, d_b2, d_ow, d_ob,
              d_ident, d_lng, d_lnb, d_out)
    nc.compile()
    return nc


def _layernorm(nc, pool, x, out, g_bc, b_bc, eps_sb, pfx):
    """LN over last dim (24) of x [128, 8, 24] -> out [128, 8, 24].

    rstd via ACT Sqrt + DVE reciprocal (keeps everything in one table set).
    """
    sums = pool.tile([128, 8], F32, name=f"{pfx}_sums", tag="ln_sums")
    nc.vector.tensor_reduce(sums, x, axis=mybir.AxisListType.X, op=OP.add)
    sumsb = _ap(sums.tensor, sums.offset,
                [sums.ap[0], list(sums.ap[1]), [0, E]])
    cent = pool.tile([128, 8, E], F32, name=f"{pfx}_cent", tag="ln_cent")
    # cent = x - sums/E
    nc.vector.scalar_tensor_tensor(cent, sumsb, -1.0 / E, x, OP.mult, OP.add)
    sq = pool.tile([128, 8, E], F32, name=f"{pfx}_sq", tag="ln_sq")
    nc.vector.tensor_tensor(sq, cent, cent, OP.mult)
    sqs = pool.tile([128, 8], F32, name=f"{pfx}_sqs", tag="ln_sqs")
    nc.vector.tensor_reduce(sqs, sq, axis=mybir.AxisListType.X, op=OP.add)
    std = pool.tile([128, 8], F32, name=f"{pfx}_std", tag="ln_std")
    nc.scalar.activation(std, sqs, AF.Sqrt, bias=eps_sb, scale=1.0 / E)
    rstd = pool.tile([128, 8], F32, name=f"{pfx}_rstd", tag="ln_rstd")
    nc.vector.reciprocal(rstd, std)
    rstdb = _ap(rstd.tensor, rstd.offset,
                [rstd.ap[0], list(rstd.ap[1]), [0, E]])
    nrm = pool.tile([128, 8, E], F32, name=f"{pfx}_nrm", tag="ln_nrm")
    nc.vector.tensor_tensor(nrm, cent, rstdb, OP.mult)
    # apply g, b (broadcast over partitions and lt): g_bc is [128, 24]
    gv = _ap(g_bc.tensor, g_bc.offset, [g_bc.ap[0], [0, 8], list(g_bc.ap[1])])
    bv = _ap(b_bc.tensor, b_bc.offset, [b_bc.ap[0], [0, 8], list(b_bc.ap[1])])
    nc.vector.tensor_tensor(nrm, nrm, gv, OP.mult)
    nc.vector.tensor_tensor(out, nrm, bv, OP.add)


def _emit(tc, nc, erf_func, d_xpad, d_cw, d_cb, d_cberf, d_peT, d_wq, d_wk,
          d_wv, d_relU, d_w1, d_b1c, d_w2, d_b2, d_ow, d_ob, d_ident,
          d_lng, d_lnb, d_out):
    from contextlib import ExitStack
    ctx = ExitStack()
    with ctx:
        singles = ctx.enter_context(tc.tile_pool(name="singles", bufs=1))
        texp_pool = ctx.enter_context(tc.tile_pool(name="texp", bufs=2))
        scratch = ctx.enter_context(tc.tile_pool(name="scratch", bufs=1))

        # ---- phase-1-critical loads first (conv + projections) ----
        xcol = singles.tile([KW, L], F32, name="xcol")
        nc.sync.dma_start(out=xcol, in_=_ap(d_xpad, 0, [[1, KW], [1, L]]))
        cw = singles.tile([KW, E], F32, name="cw_sb")
        nc.sync.dma_start(out=cw, in_=d_cw.ap())
        cb = singles.tile([E, 1], F32, name="cb_sb")
        nc.sync.dma_start(out=cb, in_=d_cb.ap())
        cberf = singles.tile([E, 1], F32, name="cberf_sb")
        nc.sync.dma_start(out=cberf, in_=d_cberf.ap())
        peT = singles.tile([E, L], F32, name="peT_sb")
        nc.sync.dma_start(out=peT, in_=d_peT.ap())
        wq = singles.tile([E, E], BF16, name="wq_sb")
        nc.sync.dma_start(out=wq, in_=d_wq.ap())
        wk = singles.tile([E, E], BF16, name="wk_sb")
        nc.sync.dma_start(out=wk, in_=d_wk.ap())
        wv = singles.tile([E, E], BF16, name="wv_sb")
        nc.sync.dma_start(out=wv, in_=d_wv.ap())
        # eRPE Toeplitz block weights, host-expanded: [128, H, 15*128] bf16
        u_all = singles.tile([128, H, 15 * 128], BF16, name="u_all")
        nc.sync.dma_start(out=u_all, in_=d_relU.ap())
        # ---- later-phase params ----
        ident = singles.tile([128, 128], F32, name="ident_sb")
        nc.sync.dma_start(out=ident, in_=d_ident.ap())
        w1 = singles.tile([E, DFF], BF16, name="w1_sb")
        nc.sync.dma_start(out=w1, in_=d_w1.ap())
        b1c = singles.tile([128, 2], F32, name="b1c_sb")
        nc.sync.dma_start(out=b1c, in_=d_b1c.ap())
        w2 = singles.tile([128, 2, E], BF16, name="w2_sb")
        nc.sync.dma_start(out=w2, in_=d_w2.ap())
        b2 = singles.tile([E, 1], F32, name="b2_sb")
        nc.sync.dma_start(out=b2, in_=d_b2.ap())
        ow = singles.tile([E, NCls], F32, name="ow_sb")
        nc.sync.dma_start(out=ow, in_=d_ow.ap())
        ob = singles.tile([NCls, 1], F32, name="ob_sb")
        nc.sync.dma_start(out=ob, in_=d_ob.ap())
        lng_bc = singles.tile([128, 3, E], F32, name="lng_bc")
        nc.sync.dma_start(out=lng_bc,
                          in_=_ap(d_lng, 0, [[0, 128], [E, 3], [1, E]]))
        lnb_bc = singles.tile([128, 3, E], F32, name="lnb_bc")
        nc.sync.dma_start(out=lnb_bc,
                          in_=_ap(d_lnb, 0, [[0, 128], [E, 3], [1, E]]))
        eps_sb = singles.tile([128, 1], F32, name="eps_sb")
        nc.vector.memset(eps_sb, EPS)
        # dummy activation: preload the erf table set before phase 1 uses it
        dummy_act = singles.tile([1, 1], F32, name="dummy_act")
        nc.vector.memset(dummy_act, 0.5)
        nc.scalar.activation(dummy_act, dummy_act, erf_func, scale=1.0)
        ones128 = singles.tile([128, 1], F32, name="ones128")
        nc.vector.memset(ones128, 1.0)
        ones11 = singles.tile([1, 1], F32, name="ones11")
        nc.vector.memset(ones11, 1.0)
        z1 = singles.tile([1, 128], F32, name="z1_sb")
        nc.vector.memset(z1, 0.0)
        z2 = singles.tile([1, 192], F32, name="z2_sb")
        nc.vector.memset(z2, 0.0)
        z512 = singles.tile([1, 512], F32, name="z512_sb")
        nc.vector.memset(z512, 0.0)

        # big single tiles
        # V in [key-in-tile, jt, head, dim|1] layout (col 3 = ones for denom)
        V_sb = singles.tile([128, 8, 8, 4], BF16, name="V_sb")
        # Q/K in 4-head-strip layout: head 4g+j at partitions 32j..32j+2
        q4 = singles.tile([128, 2, L], BF16, name="q4")
        k4 = singles.tile([128, 2, L], BF16, name="k4")
        aoT_stack = singles.tile([32, L], F32, name="aoT_stack")
        xsrcT = singles.tile([E, L], F32, name="xsrcT")
        xposT = singles.tile([E, L], F32, name="xposT")

        # ============ phase 1: conv embed + BN + GELU + tAPE ============
        with tc.tile_pool(name="ph1ps", bufs=1, space="PSUM") as ph1ps, \
             tc.tile_pool(name="prjps", bufs=2, space="PSUM") as prjps, \
             tc.tile_pool(name="ph1sb", bufs=1) as ph1sb:
            conv_ps = ph1ps.tile([E, L], F32, name="conv_ps")
            for hh in range(2):
                nc.tensor.matmul(conv_ps[:, hh * 512:(hh + 1) * 512],
                                 cw,
                                 xcol[:, hh * 512:(hh + 1) * 512],
                                 start=True, stop=True)
            # exact GELU via erf: gelu(y) = 0.5 * y * (1 + erf(y/sqrt(2)))
            e_t = ph1sb.tile([E, L], F32, name="e_t")
            nc.scalar.activation(e_t, conv_ps, erf_func, bias=cberf,
                                 scale=INV_SQRT2)
            # y_t = conv_ps + cb on DVE (parallel with erf on Scalar)
            y_t = ph1sb.tile([E, L], F32, name="y_t")
            nc.vector.tensor_scalar(y_t, conv_ps, cb, 0.0, OP.add, OP.add)
            # preload exp table set while projections run on PE
            nc.scalar.activation(dummy_act, dummy_act, AF.Exp, scale=1.0)
            tmp_g = ph1sb.tile([E, L], F32, name="tmp_g")
            nc.vector.scalar_tensor_tensor(tmp_g, e_t, 1.0, y_t,
                                           OP.add, OP.mult)
            nc.vector.tensor_scalar(xsrcT, tmp_g, 0.5, 0.0, OP.mult, OP.add)
            nc.vector.scalar_tensor_tensor(xposT, tmp_g, 0.5, peT,
                                           OP.mult, OP.add)
            xposT_bf = ph1sb.tile([E, L], BF16, name="xposT_bf")
            nc.vector.tensor_copy(xposT_bf, xposT)

            # ---- Q^T, K^T projections + per-head repack to 32-strips ----
            # head 4g+j lands at partitions 32j..32j+2 of q4/k4[:, g, :]
            for (w_, dst, nm) in ((wq, q4, "q"), (wk, k4, "k")):
                prj = prjps.tile([E, L], F32, name=f"prj_{nm}", tag="prj")
                for hh in range(2):
                    nc.tensor.matmul(prj[:, hh * 512:(hh + 1) * 512],
                                     w_,
                                     xposT_bf[:, hh * 512:(hh + 1) * 512],
                                     start=True, stop=True)
                stg = ph1sb.tile([E, L], BF16, name=f"stg_{nm}", tag="stg")
                nc.vector.tensor_copy(stg, prj)
                for h in range(H):
                    g, j = h // 4, h % 4
                    nc.sync.dma_start(out=dst[32 * j:32 * j + 3, g, :],
                                      in_=stg[3 * h:3 * h + 3, :])

            # ---- V in [key, jt, head, dim|1] layout ----
            nc.vector.memset(V_sb, 1.0)
            for jt in range(8):
                vps = ph1ps.tile([128, E], F32, name=f"vps{jt}", tag="vps")
                nc.tensor.matmul(vps,
                                 xposT_bf[:, jt * 128:(jt + 1) * 128],
                                 wv, start=True, stop=True)
                vview = _ap(vps.tensor, vps.offset, [vps.ap[0], [3, 8], [1, 3]])
                dst = _ap(V_sb.tensor, V_sb.offset + jt * 32,
                          [V_sb.ap[0], [4, 8], [1, 3]])
                nc.vector.tensor_copy(dst, vview)

        # ============ phase 2: attention (2-head pairs, pipelined) ============
        # step = (pair p of heads 2p,2p+1; query-half hh; key-tile jt).
        # S matmuls run one step AHEAD of EXP so EXP is gapless on Scalar.
        with tc.tile_pool(name="biasps", bufs=1, space="PSUM") as biasps:
            bias_ps = biasps.tile([128, H, 8, HD], F32, name="bias_ps")
            flat = bias_ps.rearrange("p a b c -> p (a b c)")
            nc.tensor.matmul(flat, z1, z2, start=True, stop=False,
                             skip_group_check=True)

            bias_work = [(h, d) for h in range(H) for d in range(-7, 8)]
            steps = [(p, hh, jt)
                     for p in range(4) for hh in range(2) for jt in range(8)]

            def emit_s(t):
                p, hh, jt = steps[t]
                s2 = sps.tile([128, 2, 512], F32, name=f"s{t}", tag="s")
                for j in range(2):
                    h = 2 * p + j
                    st = 32 * (h % 4)
                    nc.tensor.matmul(
                        s2[:, j, :],
                        k4[st:st + 3, h // 4, jt * 128:(jt + 1) * 128],
                        q4[st:st + 3, h // 4, hh * 512:(hh + 1) * 512],
                        start=True, stop=True,
                        tile_position=(st, 0),
                        skip_group_check=True)
                return s2

            with tc.tile_pool(name="sps", bufs=2, space="PSUM") as sps, \
                 tc.tile_pool(name="aops", bufs=2, space="PSUM") as aops, \
                 tc.tile_pool(name="aosb", bufs=2) as aosb_pool:
                s_cur = emit_s(0)
                ao_ps = None
                bi = 0
                for t, (p, hh, jt) in enumerate(steps):
                    texp = texp_pool.tile([128, 2, 512], BF16,
                                          name=f"tx{t}", tag="texp")
                    nc.scalar.activation(texp, s_cur, AF.Exp, scale=SCALE)
                    if t + 1 < len(steps):
                        s_cur = emit_s(t + 1)
                    if jt == 0:
                        # fresh accumulator: zero-fill whole tile so the
                        # unused partitions are initialized for the drain
                        ao_ps = aops.tile([128, 512], F32, name=f"ao{t}",
                                          tag="ao")
                        nc.tensor.matmul(ao_ps, z1, z512, start=True,
                                         stop=False, skip_group_check=True)
                    for j in range(2):
                        h = 2 * p + j
                        nc.tensor.matmul(
                            ao_ps[32 * j:32 * j + 4, :],
                            V_sb[:, jt, h, :],
                            texp[:, j, :],
                            start=False, stop=(jt == 7),
                            tile_position=(0, 32 * j),
                            skip_group_check=True)
                    # spread the 120 eRPE bias matmuls over the 64 steps
                    n_this = (120 * (t + 1)) // len(steps) - bi
                    for _ in range(n_this):
                        h, d = bias_work[bi]
                        bi += 1
                        jt0 = max(0, -d)
                        n = 8 - abs(d)
                        it0 = max(0, d)
                        nc.tensor.matmul(
                            bias_ps[:, h, it0:it0 + n, :],
                            u_all[:, h, (d + 7) * 128:(d + 8) * 128],
                            V_sb[:, jt0:jt0 + n, h, 0:3],
                            start=False, stop=False,
                            skip_group_check=True)
                    if jt == 7:
                        ao_sb = aosb_pool.tile([128, 512], F32,
                                               name=f"aosb{t}", tag="aosb")
                        nc.vector.tensor_copy(ao_sb, ao_ps)
                        for j in range(2):
                            h = 2 * p + j
                            nc.sync.dma_start(
                                out=aoT_stack[4 * h:4 * h + 4,
                                              hh * 512:(hh + 1) * 512],
                                in_=ao_sb[32 * j:32 * j + 4, :])
                nc.tensor.matmul(flat, z1, z2, start=False, stop=True,
                                 skip_group_check=True)
            # preload sqrt table set while phase-3 transposes run
            nc.scalar.activation(dummy_act, dummy_act, AF.Sqrt, scale=1.0)

            # ======== phase 3: transpose ao + z assembly ========
            z_sb = singles.tile([128, 8, E], F32, name="z_sb")
            with tc.tile_pool(name="trps", bufs=2, space="PSUM") as trps, \
                 tc.tile_pool(name="trsb", bufs=2) as trsb:
                for lt in range(8):
                    tr_ps = trps.tile([128, 32], F32, name=f"tr{lt}", tag="tr")
                    nc.tensor.transpose(tr_ps,
                                        aoT_stack[:, lt * 128:(lt + 1) * 128],
                                        ident[:32, :32])
                    tr_sb = trsb.tile([128, 8, 4], F32, name=f"trsb{lt}",
                                      tag="trs")
                    nc.vector.tensor_copy(tr_sb, tr_ps)
                    # ao = A * (1/d) + B  (d = denom col 3; B = bias_ps slice)
                    rec = trsb.tile([128, 8], F32, name=f"rec{lt}", tag="rec")
                    nc.vector.reciprocal(rec, tr_sb[:, :, 3])
                    recb = _ap(rec.tensor, rec.offset,
                               [rec.ap[0], list(rec.ap[1]), [0, 3]])
                    an = trsb.tile([128, 8, 3], F32, name=f"an{lt}", tag="an")
                    nc.vector.tensor_tensor(an, tr_sb[:, :, 0:3], recb,
                                            OP.mult)
                    nc.vector.tensor_tensor(z_sb[:, lt, :].rearrange(
                        "p (a b) -> p a b", a=8), an, bias_ps[:, :, lt, :],
                        OP.add)

        # ======== phase 4: LNs + FFN + pool + head ========
        y1 = singles.tile([128, 8, E], F32, name="y1_sb")
        att_L = singles.tile([128, 8, E], F32, name="attL_sb")
        y2 = singles.tile([128, 8, E], F32, name="y2_sb")
        out_L = singles.tile([128, 8, E], F32, name="outL_sb")
        zln = singles.tile([128, 8, E], F32, name="zln_sb")
        attT = singles.tile([E, L], BF16, name="attT_sb")
        ffh0 = singles.tile([128, L], BF16, name="ffh0_sb")
        ffh1 = singles.tile([128, L], BF16, name="ffh1_sb")
        ffT = singles.tile([E, L], F32, name="ffT_sb")

        _layernorm(nc, scratch, z_sb, zln, lng_bc[:, 0, :], lnb_bc[:, 0, :],
                   eps_sb, "aln")
        with tc.tile_pool(name="xsps", bufs=2, space="PSUM") as xsps:
            for lt in range(8):
                xs_ps = xsps.tile([128, E], F32, name=f"xs{lt}", tag="xs")
                nc.tensor.transpose(xs_ps, xsrcT[:, lt * 128:(lt + 1) * 128],
                                    ident[:E, :E])
                nc.vector.tensor_tensor(y1[:, lt, :], zln[:, lt, :], xs_ps,
                                        OP.add)
        _layernorm(nc, scratch, y1, att_L, lng_bc[:, 1, :], lnb_bc[:, 1, :],
                   eps_sb, "ln1")

        with tc.tile_pool(name="atps", bufs=1, space="PSUM") as atps:
            attT_ps = atps.tile([E, L], F32, name="attT_ps")
            for lt in range(8):
                nc.tensor.transpose(attT_ps[:, lt * 128:(lt + 1) * 128],
                                    att_L[:, lt, :], ident)
            nc.vector.tensor_copy(attT, attT_ps)

        with tc.tile_pool(name="ffps", bufs=2, space="PSUM") as ffps:
            for p2, ffh in ((0, ffh0), (1, ffh1)):
                ffh_ps = ffps.tile([128, L], F32, name=f"ffh{p2}", tag="ffh")
                for hh in range(2):
                    nc.tensor.matmul(ffh_ps[:, hh * 512:(hh + 1) * 512],
                                     w1[:, p2 * 128:(p2 + 1) * 128],
                                     attT[:, hh * 512:(hh + 1) * 512],
                                     start=True, stop=True)
                nc.scalar.activation(ffh, ffh_ps, AF.Relu,
                                     bias=b1c[:, p2:p2 + 1], scale=1.0)

        with tc.tile_pool(name="f2ps", bufs=1, space="PSUM") as f2ps:
            ffT_ps = f2ps.tile([E, L], F32, name="ffT_ps")
            for hh in range(2):
                for p2, ffh in ((0, ffh0), (1, ffh1)):
                    nc.tensor.matmul(
                        ffT_ps[:, hh * 512:(hh + 1) * 512],
                        w2[:, p2, :],
                        ffh[:, hh * 512:(hh + 1) * 512],
                        start=(p2 == 0), stop=(p2 == 1))
            nc.scalar.activation(ffT, ffT_ps, AF.Identity, bias=b2, scale=1.0)

        with tc.tile_pool(name="fmps", bufs=2, space="PSUM") as fmps:
            for lt in range(8):
                ff_ps = fmps.tile([128, E], F32, name=f"ffm{lt}", tag="ffm")
                nc.tensor.transpose(ff_ps, ffT[:, lt * 128:(lt + 1) * 128],
                                    ident[:E, :E])
                nc.vector.tensor_tensor(y2[:, lt, :], att_L[:, lt, :], ff_ps,
                                        OP.add)
        _layernorm(nc, scratch, y2, out_L, lng_bc[:, 2, :], lnb_bc[:, 2, :],
                   eps_sb, "ln2")

        with tc.tile_pool(name="hdps", bufs=1, space="PSUM") as hdps, \
             tc.tile_pool(name="hdsb", bufs=1) as hdsb:
            pooled_ps = hdps.tile([1, E], F32, name="pooled_ps")
            for lt in range(8):
                nc.tensor.matmul(pooled_ps, ones128, out_L[:, lt, :],
                                 start=(lt == 0), stop=(lt == 7))
            pooled_sb = hdsb.tile([1, E], F32, name="pooled_sb")
            nc.vector.tensor_copy(pooled_sb, pooled_ps)
            pooledT_ps = hdps.tile([E, 1], F32, name="pooledT_ps")
            nc.tensor.matmul(pooledT_ps, pooled_sb, ones11, start=True,
                             stop=True)
            pooledT_sb = hdsb.tile([E, 1], F32, name="pooledT_sb")
            nc.vector.tensor_copy(pooledT_sb, pooledT_ps)
            logits_ps = hdps.tile([NCls, 1], F32, name="logits_ps")
            nc.tensor.matmul(logits_ps, ow, pooledT_sb, start=True, stop=True)
            logits_sb = hdsb.tile([NCls, 1], F32, name="logits_sb")
            nc.scalar.activation(logits_sb, logits_ps, AF.Identity, bias=ob,
                                 scale=1.0 / L)
            nc.sync.dma_start(out=d_out.ap(), in_=logits_sb)


def host_prep(inputs, erf=None):
    """Host-side parameter prep (tiny, O(E*K)). Returns (shared, per_core)."""
    f32 = np.float32
    a = (inputs["bn_gamma"] / np.sqrt(inputs["bn_var"] + EPS)).astype(f32)
    cw = (inputs["conv_w"][:, 0, :].T * a[None, :]).astype(f32)  # [K, E]
    cb = ((inputs["conv_b"] - inputs["bn_mean"]) * a
          + inputs["bn_beta"]).astype(f32).reshape(E, 1)
    # tAPE positional encoding
    pos = np.arange(L, dtype=f32)[:, None]
    div = np.exp(np.arange(0, E, 2, dtype=f32) * (-math.log(10000.0) / E))
    ang = pos * div * (float(E) / float(L))
    pe = np.zeros((L, E), f32)
    pe[:, 0::2] = np.sin(ang)
    pe[:, 1::2] = np.cos(ang)
    b1 = inputs["ff_b1"].astype(f32)
    b1c = np.stack([b1[:128], b1[128:]], axis=1)  # [128, 2]
    shared = {
        "cw": cw,
        "cb": cb,
        "cberf": (cb * INV_SQRT2).astype(f32),
        "peT": pe.T.copy(),
        "wq": inputs["wq"].astype(f32).astype(mybir.dt.np(BF16)),
        "wk": inputs["wk"].astype(f32).astype(mybir.dt.np(BF16)),
        "wv": inputs["wv"].astype(f32).astype(mybir.dt.np(BF16)),
        # eRPE Toeplitz blocks, expanded: U[j', h, m] = table[127 - j' + m, h]
        "relU": np.ascontiguousarray(
            inputs["rel_bias_table"].astype(f32)[
                127 - np.arange(128)[:, None] + np.arange(15 * 128)[None, :]
            ].transpose(0, 2, 1)).astype(mybir.dt.np(BF16)),
        "w1": inputs["ff_w1"].astype(f32).astype(mybir.dt.np(BF16)),
        "b1c": b1c.copy(),
        "w2": np.ascontiguousarray(
            inputs["ff_w2"].astype(f32).reshape(2, 128, E).transpose(
                1, 0, 2)).astype(mybir.dt.np(BF16)),
        "b2": inputs["ff_b2"].astype(f32).reshape(E, 1),
        "ow": inputs["out_w"].astype(f32),
        "ob": inputs["out_b"].astype(f32).reshape(NCls, 1),
        "ident": np.eye(128, dtype=f32),
        "lng": np.stack([inputs["attn_ln_g"], inputs["ln1_g"],
                         inputs["ln2_g"]]).astype(f32),
        "lnb": np.stack([inputs["attn_ln_b"], inputs["ln1_b"],
                         inputs["ln2_b"]]).astype(f32),
    }
    x = inputs["x"].astype(f32)  # (B, 1, L)
    per_core = []
    for b in range(B):
        xpad = np.zeros((L + KW - 1,), f32)
        xpad[3:3 + L] = x[b, 0]
        per_core.append({"xpad": xpad, **shared})
    return per_core


_NC_CACHE = {}


def kernel(**inputs) -> np.ndarray:
    from concourse.bass_utils import run_bass_kernel_spmd
    if "nc" not in _NC_CACHE:
        _NC_CACHE["nc"] = build_nc()
    nc = _NC_CACHE["nc"]
    in_maps = host_prep(inputs)
    res = run_bass_kernel_spmd(nc, in_maps, core_ids=list(range(NCORES)))
    out = np.stack([res.results[b]["out"].reshape(NCls) for b in range(B)])
    return out.astype(np.float32)


if __name__ == "__main__":
    import reference
    ins = {k: np.asarray(v) for k, v in reference.setup_inputs().items()}
    got = kernel(**ins)
    exp = np.asarray(reference.reference(**reference.setup_inputs()))
    err = np.abs(got - exp).max() / np.abs(exp).max()
    print("Relative error:", err)


# revision 36
# speedup vs baseline: 14.6096x; 1.1283x over previous
"""Bass/Tile TRN2 kernel for nn_ConvTran_618475290811.

ConvTran tiny transformer: conv embed + BN + GELU + tAPE + eRPE attention
(bias added AFTER softmax) + FFN + mean-pool + classifier head.
B=8 batch elements, data-parallel one per NeuronCore (8 cores).

Key tricks:
 - attention computed in transposed (S^T = [keys, queries]) layout; softmax
   denominator produced for free via a ones-column appended to V.
 - no division for softmax: LayerNorm scale-invariance lets us feed
   z = exp@v + denom * (R@v) into the to_out LayerNorm.
 - eRPE Toeplitz bias R@v via 15 diagonal-block stationary weights per head,
   host-expanded into a [128, H, 15*128] bf16 tensor (single contiguous DMA).
 - 4-head PE packing: S matmuls (K=3) row-tiled at tile_position=(32j, 0),
   AO matmuls (M=4) col-tiled at tile_position=(0, 32j) - 4 run concurrently.
 - one EXP per (group, key-tile, query-half) over N=2048 (4 PSUM banks).
 - LayerNorm rstd via Sqrt + DVE reciprocal (single ACT table set in tail).
"""
import math
import numpy as np

import concourse.bass as bass
import concourse.bacc as bacc
import concourse.tile as tile
from concourse import mybir

B, L, E, H, NCls, DFF, KW = 8, 1024, 24, 8, 10, 256, 8
HD = E // H  # 3
NCORES = 8
F32 = mybir.dt.float32
BF16 = mybir.dt.bfloat16
AF = mybir.ActivationFunctionType
OP = mybir.AluOpType
SCALE = float(E) ** -0.5
INV_SQRT2 = 0.7071067811865476
EPS = 1e-5


def _ap(t, off, pattern):
    return bass.AP(t, off, pattern)


def build_nc(erf_func=AF.Erf):
    nc = bacc.Bacc("TRN2", target_bir_lowering=False, debug=False)

    # ---- DRAM I/O ----
    d_xpad = nc.dram_tensor("xpad", [L + KW - 1], F32, kind="ExternalInput")
    d_cw = nc.dram_tensor("cw", [KW, E], F32, kind="ExternalInput")
    d_cb = nc.dram_tensor("cb", [E, 1], F32, kind="ExternalInput")
    d_cberf = nc.dram_tensor("cberf", [E, 1], F32, kind="ExternalInput")
    d_peT = nc.dram_tensor("peT", [E, L], F32, kind="ExternalInput")
    d_wq = nc.dram_tensor("wq", [E, 2, 128], BF16, kind="ExternalInput")
    d_wk = nc.dram_tensor("wk", [E, 2, 128], BF16, kind="ExternalInput")
    d_wv = nc.dram_tensor("wv", [E, E], BF16, kind="ExternalInput")
    d_relU = nc.dram_tensor("relU", [128, H, 15 * 128], BF16,
                            kind="ExternalInput")
    d_w1 = nc.dram_tensor("w1", [E, DFF], BF16, kind="ExternalInput")
    d_b1c = nc.dram_tensor("b1c", [128, 2], F32, kind="ExternalInput")
    d_w2 = nc.dram_tensor("w2", [128, 2, E], BF16, kind="ExternalInput")
    d_b2 = nc.dram_tensor("b2", [E, 1], F32, kind="ExternalInput")
    d_ow = nc.dram_tensor("ow", [E, NCls], F32, kind="ExternalInput")
    d_ob = nc.dram_tensor("ob", [NCls, 1], F32, kind="ExternalInput")
    d_ident = nc.dram_tensor("ident", [128, 128], F32, kind="ExternalInput")
    # 6 LayerNorm gain/bias rows: attn_ln, ln1, ln2
    d_lng = nc.dram_tensor("lng", [3, E], F32, kind="ExternalInput")
    d_lnb = nc.dram_tensor("lnb", [3, E], F32, kind="ExternalInput")
    d_out = nc.dram_tensor("out", [NCls, 1], F32, kind="ExternalOutput")

    with tile.TileContext(nc) as tc:
        _emit(tc, nc, erf_func, d_xpad, d_cw, d_cb, d_cberf, d_peT, d_wq,
              d_wk, d_wv, d_relU, d_w1, d_b1c, d_w2, d_b2, d_ow, d_ob,
              d_ident, d_lng, d_lnb, d_out)
    nc.compile()
    return nc


def _layernorm(nc, pool, x, out, g_bc, b_bc, eps_sb, pfx):
    """LN over last dim (24) of x [128, 8, 24] -> out [128, 8, 24].

    rstd via ACT Sqrt + DVE reciprocal (keeps everything in one table set).
    """
    sums = pool.tile([128, 8], F32, name=f"{pfx}_sums", tag="ln_sums")
    nc.vector.tensor_reduce(sums, x, axis=mybir.AxisListType.X, op=OP.add)
    sumsb = _ap(sums.tensor, sums.offset,
                [sums.ap[0], list(sums.ap[1]), [0, E]])
    cent = pool.tile([128, 8, E], F32, name=f"{pfx}_cent", tag="ln_cent")
    # cent = x - sums/E
    nc.vector.scalar_tensor_tensor(cent, sumsb, -1.0 / E, x, OP.mult, OP.add)
    sq = pool.tile([128, 8, E], F32, name=f"{pfx}_sq", tag="ln_sq")
    nc.vector.tensor_tensor(sq, cent, cent, OP.mult)
    sqs = pool.tile([128, 8], F32, name=f"{pfx}_sqs", tag="ln_sqs")
    nc.vector.tensor_reduce(sqs, sq, axis=mybir.AxisListType.X, op=OP.add)
    std = pool.tile([128, 8], F32, name=f"{pfx}_std", tag="ln_std")
    nc.scalar.activation(std, sqs, AF.Sqrt, bias=eps_sb, scale=1.0 / E)
    rstd = pool.tile([128, 8], F32, name=f"{pfx}_rstd", tag="ln_rstd")
    nc.vector.reciprocal(rstd, std)
    rstdb = _ap(rstd.tensor, rstd.offset,
                [rstd.ap[0], list(rstd.ap[1]), [0, E]])
    nrm = pool.tile([128, 8, E], F32, name=f"{pfx}_nrm", tag="ln_nrm")
    nc.vector.tensor_tensor(nrm, cent, rstdb, OP.mult)
    # apply g, b (broadcast over partitions and lt): g_bc is [128, 24]
    gv = _ap(g_bc.tensor, g_bc.offset, [g_bc.ap[0], [0, 8], list(g_bc.ap[1])])
    bv = _ap(b_bc.tensor, b_bc.offset, [b_bc.ap[0], [0, 8], list(b_bc.ap[1])])
    nc.vector.tensor_tensor(nrm, nrm, gv, OP.mult)
    nc.vector.tensor_tensor(out, nrm, bv, OP.add)


def _emit(tc, nc, erf_func, d_xpad, d_cw, d_cb, d_cberf, d_peT, d_wq, d_wk,
          d_wv, d_relU, d_w1, d_b1c, d_w2, d_b2, d_ow, d_ob, d_ident,
          d_lng, d_lnb, d_out):
    from contextlib import ExitStack
    ctx = ExitStack()
    with ctx:
        singles = ctx.enter_context(tc.tile_pool(name="singles", bufs=1))
        texp_pool = ctx.enter_context(tc.tile_pool(name="texp", bufs=2))
        scratch = ctx.enter_context(tc.tile_pool(name="scratch", bufs=1))

        # ---- phase-1-critical loads first (conv + projections) ----
        xcol = singles.tile([KW, L], F32, name="xcol")
        nc.sync.dma_start(out=xcol, in_=_ap(d_xpad, 0, [[1, KW], [1, L]]))
        cw = singles.tile([KW, E], F32, name="cw_sb")
        nc.sync.dma_start(out=cw, in_=d_cw.ap())
        cb = singles.tile([E, 1], F32, name="cb_sb")
        nc.sync.dma_start(out=cb, in_=d_cb.ap())
        cberf = singles.tile([E, 1], F32, name="cberf_sb")
        nc.sync.dma_start(out=cberf, in_=d_cberf.ap())
        peT = singles.tile([E, L], F32, name="peT_sb")
        nc.sync.dma_start(out=peT, in_=d_peT.ap())
        wq = singles.tile([E, 2, 128], BF16, name="wq_sb")
        nc.sync.dma_start(out=wq, in_=d_wq.ap())
        wk = singles.tile([E, 2, 128], BF16, name="wk_sb")
        nc.sync.dma_start(out=wk, in_=d_wk.ap())
        wv = singles.tile([E, E], BF16, name="wv_sb")
        nc.sync.dma_start(out=wv, in_=d_wv.ap())
        # eRPE Toeplitz block weights, host-expanded: [128, H, 15*128] bf16
        u_all = singles.tile([128, H, 15 * 128], BF16, name="u_all")
        nc.sync.dma_start(out=u_all, in_=d_relU.ap())
        # ---- later-phase params ----
        ident = singles.tile([128, 128], F32, name="ident_sb")
        nc.sync.dma_start(out=ident, in_=d_ident.ap())
        w1 = singles.tile([E, DFF], BF16, name="w1_sb")
        nc.sync.dma_start(out=w1, in_=d_w1.ap())
        b1c = singles.tile([128, 2], F32, name="b1c_sb")
        nc.sync.dma_start(out=b1c, in_=d_b1c.ap())
        w2 = singles.tile([128, 2, E], BF16, name="w2_sb")
        nc.sync.dma_start(out=w2, in_=d_w2.ap())
        b2 = singles.tile([E, 1], F32, name="b2_sb")
        nc.sync.dma_start(out=b2, in_=d_b2.ap())
        ow = singles.tile([E, NCls], F32, name="ow_sb")
        nc.sync.dma_start(out=ow, in_=d_ow.ap())
        ob = singles.tile([NCls, 1], F32, name="ob_sb")
        nc.sync.dma_start(out=ob, in_=d_ob.ap())
        lng_bc = singles.tile([128, 3, E], F32, name="lng_bc")
        nc.sync.dma_start(out=lng_bc,
                          in_=_ap(d_lng, 0, [[0, 128], [E, 3], [1, E]]))
        lnb_bc = singles.tile([128, 3, E], F32, name="lnb_bc")
        nc.sync.dma_start(out=lnb_bc,
                          in_=_ap(d_lnb, 0, [[0, 128], [E, 3], [1, E]]))
        eps_sb = singles.tile([128, 1], F32, name="eps_sb")
        nc.vector.memset(eps_sb, EPS)
        # dummy activation: preload the erf table set before phase 1 uses it
        dummy_act = singles.tile([1, 1], F32, name="dummy_act")
        nc.vector.memset(dummy_act, 0.5)
        nc.scalar.activation(dummy_act, dummy_act, erf_func, scale=1.0)
        ones128 = singles.tile([128, 1], F32, name="ones128")
        nc.vector.memset(ones128, 1.0)
        ones11 = singles.tile([1, 1], F32, name="ones11")
        nc.vector.memset(ones11, 1.0)
        z1 = singles.tile([1, 128], F32, name="z1_sb")
        nc.vector.memset(z1, 0.0)
        z2 = singles.tile([1, 192], F32, name="z2_sb")
        nc.vector.memset(z2, 0.0)


        # big single tiles
        # V in [key-in-tile, jt, head, dim|1] layout (col 3 = ones for denom)
        V_sb = singles.tile([128, 8, 8, 4], BF16, name="V_sb")
        # Q/K in 4-head-strip layout: head 4g+j at partitions 32j..32j+2
        q4 = singles.tile([128, 2, L], BF16, name="q4")
        k4 = singles.tile([128, 2, L], BF16, name="k4")
        aoT_stack = singles.tile([32, L], F32, name="aoT_stack")
        xsrcT = singles.tile([E, L], F32, name="xsrcT")

        # ============ phase 1: conv embed + BN + GELU + tAPE ============
        with tc.tile_pool(name="ph1ps", bufs=1, space="PSUM") as ph1ps, \
             tc.tile_pool(name="prjps", bufs=2, space="PSUM") as prjps, \
             tc.tile_pool(name="ph1sb", bufs=1) as ph1sb:
            conv_ps = ph1ps.tile([E, L], F32, name="conv_ps")
            for hh in range(2):
                nc.tensor.matmul(conv_ps[:, hh * 512:(hh + 1) * 512],
                                 cw,
                                 xcol[:, hh * 512:(hh + 1) * 512],
                                 start=True, stop=True)
            # exact GELU via erf: gelu(y) = 0.5 * y * (1 + erf(y/sqrt(2)))
            e_t = ph1sb.tile([E, L], F32, name="e_t")
            nc.scalar.activation(e_t, conv_ps, erf_func, bias=cberf,
                                 scale=INV_SQRT2)
            # y_t = conv_ps + cb on DVE (parallel with erf on Scalar)
            y_t = ph1sb.tile([E, L], F32, name="y_t")
            nc.vector.tensor_scalar(y_t, conv_ps, cb, 0.0, OP.add, OP.add)
            # preload exp table set while projections run on PE; the e_t
            # read anchors this after the erf (no scheduler hoist)
            nc.scalar.activation(dummy_act, e_t[0:1, 0:1], AF.Exp, scale=1.0)
            tmp_g = ph1sb.tile([E, L], F32, name="tmp_g")
            nc.vector.scalar_tensor_tensor(tmp_g, e_t, 1.0, y_t,
                                           OP.add, OP.mult)
            xposT_bf = ph1sb.tile([E, L], BF16, name="xposT_bf")
            nc.vector.scalar_tensor_tensor(xposT_bf, tmp_g, 0.5, peT,
                                           OP.mult, OP.add)
            # xsrcT (residual path) is only needed in phase 4 - off the
            # critical path into attention
            nc.vector.tensor_scalar(xsrcT, tmp_g, 0.5, 0.0, OP.mult, OP.add)

            # ---- Q^T, K^T projections, strip layout via padded weights ----
            # wq/wk host-padded to [E, 2, 128]: head 4g+j at cols 32j..32j+2.
            # One matmul per (tensor, g, hh) writes all 128 partitions with
            # head data in the 32-strips the attention matmuls expect.
            for (w_, dst, nm) in ((wq, q4, "q"), (wk, k4, "k")):
                for g in range(2):
                    for hh in range(2):
                        prj = prjps.tile([128, 512], F32,
                                         name=f"prj_{nm}{g}{hh}", tag="prj")
                        nc.tensor.matmul(prj, w_[:, g, :],
                                         xposT_bf[:, hh * 512:(hh + 1) * 512],
                                         start=True, stop=True)
                        nc.vector.tensor_copy(
                            dst[:, g, hh * 512:(hh + 1) * 512], prj)

            # ---- V in [key, jt, head, dim|1] layout ----
            nc.vector.memset(V_sb, 1.0)
            for jt in range(8):
                vps = prjps.tile([128, E], F32, name=f"vps{jt}", tag="vps")
                nc.tensor.matmul(vps,
                                 xposT_bf[:, jt * 128:(jt + 1) * 128],
                                 wv, start=True, stop=True)
                vview = _ap(vps.tensor, vps.offset, [vps.ap[0], [3, 8], [1, 3]])
                dst = _ap(V_sb.tensor, V_sb.offset + jt * 32,
                          [V_sb.ap[0], [4, 8], [1, 3]])
                nc.vector.tensor_copy(dst, vview)

        # ============ phase 2: attention (2-head pairs, pipelined) ============
        # step = (pair p of heads 2p,2p+1; query-half hh; key-tile jt).
        # S matmuls run one step AHEAD of EXP so EXP is gapless on Scalar.
        with tc.tile_pool(name="biasps", bufs=1, space="PSUM") as biasps:
            bias_ps = biasps.tile([128, H, 8, HD], F32, name="bias_ps")
            flat = bias_ps.rearrange("p a b c -> p (a b c)")
            nc.tensor.matmul(flat, z1, z2, start=True, stop=False,
                             skip_group_check=True)

            bias_work = [(h, d) for h in range(H) for d in range(-7, 8)]
            steps = [(p, hh, jt)
                     for p in range(4) for hh in range(2) for jt in range(8)]

            def emit_s(t):
                p, hh, jt = steps[t]
                s2 = sps.tile([128, 2, 512], F32, name=f"s{t}", tag="s")
                for j in range(2):
                    h = 2 * p + j
                    st = 32 * (h % 4)
                    nc.tensor.matmul(
                        s2[:, j, :],
                        k4[st:st + 3, h // 4, jt * 128:(jt + 1) * 128],
                        q4[st:st + 3, h // 4, hh * 512:(hh + 1) * 512],
                        start=True, stop=True,
                        tile_position=(st, 0),
                        skip_group_check=True)
                return s2

            with tc.tile_pool(name="sps", bufs=2, space="PSUM") as sps, \
                 tc.tile_pool(name="aops", bufs=2, space="PSUM") as aops, \
                 tc.tile_pool(name="aosb", bufs=2) as aosb_pool:
                s_cur = emit_s(0)
                ao_ps = None
                bi = 0
                for t, (p, hh, jt) in enumerate(steps):
                    texp = texp_pool.tile([128, 2, 512], BF16,
                                          name=f"tx{t}", tag="texp")
                    nc.scalar.activation(texp, s_cur, AF.Exp, scale=SCALE)
                    if t + 1 < len(steps):
                        s_cur = emit_s(t + 1)
                    if jt == 0:
                        ao_ps = aops.tile([128, 512], F32, name=f"ao{t}",
                                          tag="ao")
                    for j in range(2):
                        h = 2 * p + j
                        nc.tensor.matmul(
                            ao_ps[32 * j:32 * j + 4, :],
                            V_sb[:, jt, h, :],
                            texp[:, j, :],
                            start=(jt == 0), stop=(jt == 7),
                            tile_position=(0, 32 * j),
                            skip_group_check=True)
                    # spread the 120 eRPE bias matmuls over the 64 steps
                    n_this = (120 * (t + 1)) // len(steps) - bi
                    for _ in range(n_this):
                        h, d = bias_work[bi]
                        bi += 1
                        jt0 = max(0, -d)
                        n = 8 - abs(d)
                        it0 = max(0, d)
                        nc.tensor.matmul(
                            bias_ps[:, h, it0:it0 + n, :],
                            u_all[:, h, (d + 7) * 128:(d + 8) * 128],
                            V_sb[:, jt0:jt0 + n, h, 0:3],
                            start=False, stop=False,
                            skip_group_check=True)
                    if jt == 7:
                        ao_sb = aosb_pool.tile([128, 512], F32,
                                               name=f"aosb{t}", tag="aosb")
                        for j in range(2):
                            h = 2 * p + j
                            nc.vector.tensor_copy(
                                ao_sb[32 * j:32 * j + 4, :],
                                ao_ps[32 * j:32 * j + 4, :])
                            nc.sync.dma_start(
                                out=aoT_stack[4 * h:4 * h + 4,
                                              hh * 512:(hh + 1) * 512],
                                in_=ao_sb[32 * j:32 * j + 4, :])
                    last_texp = texp
                nc.tensor.matmul(flat, z1, z2, start=False, stop=True,
                                 skip_group_check=True)
            # preload sqrt table set while phase-3 transposes run; reading
            # from the last texp anchors this AFTER the attention exps so
            # the scheduler cannot hoist the table swap earlier
            nc.scalar.activation(dummy_act, last_texp[0:1, 0, 0:1], AF.Sqrt,
                                 scale=1.0)

            # ======== phase 3: transpose ao + z assembly ========
            z_sb = singles.tile([128, 8, E], F32, name="z_sb")
            with tc.tile_pool(name="trps", bufs=2, space="PSUM") as trps, \
                 tc.tile_pool(name="trsb", bufs=2) as trsb:
                for lt in range(8):
                    tr_ps = trps.tile([128, 32], F32, name=f"tr{lt}", tag="tr")
                    nc.tensor.transpose(tr_ps,
                                        aoT_stack[:, lt * 128:(lt + 1) * 128],
                                        ident[:32, :32])
                    tr_sb = trsb.tile([128, 8, 4], F32, name=f"trsb{lt}",
                                      tag="trs")
                    nc.vector.tensor_copy(tr_sb, tr_ps)
                    # ao = A * (1/d) + B  (d = denom col 3; B = bias_ps slice)
                    rec = trsb.tile([128, 8], F32, name=f"rec{lt}", tag="rec")
                    nc.vector.reciprocal(rec, tr_sb[:, :, 3])
                    recb = _ap(rec.tensor, rec.offset,
                               [rec.ap[0], list(rec.ap[1]), [0, 3]])
                    an = trsb.tile([128, 8, 3], F32, name=f"an{lt}", tag="an")
                    nc.vector.tensor_tensor(an, tr_sb[:, :, 0:3], recb,
                                            OP.mult)
                    nc.vector.tensor_tensor(z_sb[:, lt, :].rearrange(
                        "p (a b) -> p a b", a=8), an, bias_ps[:, :, lt, :],
                        OP.add)

        # ======== phase 4: LNs + FFN + pool + head ========
        y1 = singles.tile([128, 8, E], F32, name="y1_sb")
        att_L = singles.tile([128, 8, E], F32, name="attL_sb")
        y2 = singles.tile([128, 8, E], F32, name="y2_sb")
        out_L = singles.tile([128, 8, E], F32, name="outL_sb")
        zln = singles.tile([128, 8, E], F32, name="zln_sb")
        attT = singles.tile([E, L], BF16, name="attT_sb")
        ffh0 = singles.tile([128, L], BF16, name="ffh0_sb")
        ffh1 = singles.tile([128, L], BF16, name="ffh1_sb")
        ffT = singles.tile([E, L], F32, name="ffT_sb")

        _layernorm(nc, scratch, z_sb, zln, lng_bc[:, 0, :], lnb_bc[:, 0, :],
                   eps_sb, "aln")
        with tc.tile_pool(name="xsps", bufs=2, space="PSUM") as xsps:
            for lt in range(8):
                xs_ps = xsps.tile([128, E], F32, name=f"xs{lt}", tag="xs")
                nc.tensor.transpose(xs_ps, xsrcT[:, lt * 128:(lt + 1) * 128],
                                    ident[:E, :E])
                nc.vector.tensor_tensor(y1[:, lt, :], zln[:, lt, :], xs_ps,
                                        OP.add)
        _layernorm(nc, scratch, y1, att_L, lng_bc[:, 1, :], lnb_bc[:, 1, :],
                   eps_sb, "ln1")

        with tc.tile_pool(name="atps", bufs=1, space="PSUM") as atps:
            attT_ps = atps.tile([E, L], F32, name="attT_ps")
            for lt in range(8):
                nc.tensor.transpose(attT_ps[:, lt * 128:(lt + 1) * 128],
                                    att_L[:, lt, :], ident)
            nc.vector.tensor_copy(attT, attT_ps)

        with tc.tile_pool(name="ffps", bufs=2, space="PSUM") as ffps:
            for p2, ffh in ((0, ffh0), (1, ffh1)):
                ffh_ps = ffps.tile([128, L], F32, name=f"ffh{p2}", tag="ffh")
                for hh in range(2):
                    nc.tensor.matmul(ffh_ps[:, hh * 512:(hh + 1) * 512],
                                     w1[:, p2 * 128:(p2 + 1) * 128],
                                     attT[:, hh * 512:(hh + 1) * 512],
                                     start=True, stop=True)
                nc.scalar.activation(ffh, ffh_ps, AF.Relu,
                                     bias=b1c[:, p2:p2 + 1], scale=1.0)

        with tc.tile_pool(name="f2ps", bufs=1, space="PSUM") as f2ps:
            ffT_ps = f2ps.tile([E, L], F32, name="ffT_ps")
            for hh in range(2):
                for p2, ffh in ((0, ffh0), (1, ffh1)):
                    nc.tensor.matmul(
                        ffT_ps[:, hh * 512:(hh + 1) * 512],
                        w2[:, p2, :],
                        ffh[:, hh * 512:(hh + 1) * 512],
                        start=(p2 == 0), stop=(p2 == 1))
            nc.scalar.activation(ffT, ffT_ps, AF.Identity, bias=b2, scale=1.0)

        with tc.tile_pool(name="fmps", bufs=2, space="PSUM") as fmps:
            for lt in range(8):
                ff_ps = fmps.tile([128, E], F32, name=f"ffm{lt}", tag="ffm")
                nc.tensor.transpose(ff_ps, ffT[:, lt * 128:(lt + 1) * 128],
                                    ident[:E, :E])
                nc.vector.tensor_tensor(y2[:, lt, :], att_L[:, lt, :], ff_ps,
                                        OP.add)
        _layernorm(nc, scratch, y2, out_L, lng_bc[:, 2, :], lnb_bc[:, 2, :],
                   eps_sb, "ln2")

        with tc.tile_pool(name="hdps", bufs=1, space="PSUM") as hdps, \
             tc.tile_pool(name="hdsb", bufs=1) as hdsb:
            pooled_ps = hdps.tile([1, E], F32, name="pooled_ps")
            for lt in range(8):
                nc.tensor.matmul(pooled_ps, ones128, out_L[:, lt, :],
                                 start=(lt == 0), stop=(lt == 7))
            pooled_sb = hdsb.tile([1, E], F32, name="pooled_sb")
            nc.vector.tensor_copy(pooled_sb, pooled_ps)
            pooledT_ps = hdps.tile([E, 1], F32, name="pooledT_ps")
            nc.tensor.matmul(pooledT_ps, pooled_sb, ones11, start=True,
                             stop=True)
            pooledT_sb = hdsb.tile([E, 1], F32, name="pooledT_sb")
            nc.vector.tensor_copy(pooledT_sb, pooledT_ps)
            logits_ps = hdps.tile([NCls, 1], F32, name="logits_ps")
            nc.tensor.matmul(logits_ps, ow, pooledT_sb, start=True, stop=True)
            logits_sb = hdsb.tile([NCls, 1], F32, name="logits_sb")
            nc.scalar.activation(logits_sb, logits_ps, AF.Identity, bias=ob,
                                 scale=1.0 / L)
            nc.sync.dma_start(out=d_out.ap(), in_=logits_sb)


def _pad_qk(w):
    """[E, E] -> [E, 2, 128] bf16; head 4g+j at cols 32j..32j+2 of slot g."""
    wp = np.zeros((E, 2, 128), np.float32)
    for h in range(H):
        g, j = h // 4, h % 4
        wp[:, g, 32 * j:32 * j + 3] = w[:, 3 * h:3 * h + 3]
    return wp.astype(mybir.dt.np(BF16))


def host_prep(inputs, erf=None):
    """Host-side parameter prep (tiny, O(E*K)). Returns (shared, per_core)."""
    f32 = np.float32
    a = (inputs["bn_gamma"] / np.sqrt(inputs["bn_var"] + EPS)).astype(f32)
    cw = (inputs["conv_w"][:, 0, :].T * a[None, :]).astype(f32)  # [K, E]
    cb = ((inputs["conv_b"] - inputs["bn_mean"]) * a
          + inputs["bn_beta"]).astype(f32).reshape(E, 1)
    # tAPE positional encoding
    pos = np.arange(L, dtype=f32)[:, None]
    div = np.exp(np.arange(0, E, 2, dtype=f32) * (-math.log(10000.0) / E))
    ang = pos * div * (float(E) / float(L))
    pe = np.zeros((L, E), f32)
    pe[:, 0::2] = np.sin(ang)
    pe[:, 1::2] = np.cos(ang)
    b1 = inputs["ff_b1"].astype(f32)
    b1c = np.stack([b1[:128], b1[128:]], axis=1)  # [128, 2]
    shared = {
        "cw": cw,
        "cb": cb,
        "cberf": (cb * INV_SQRT2).astype(f32),
        "peT": pe.T.copy(),
        "wq": _pad_qk(inputs["wq"].astype(f32)),
        "wk": _pad_qk(inputs["wk"].astype(f32)),
        "wv": inputs["wv"].astype(f32).astype(mybir.dt.np(BF16)),
        # eRPE Toeplitz blocks, expanded: U[j', h, m] = table[127 - j' + m, h]
        "relU": np.ascontiguousarray(
            inputs["rel_bias_table"].astype(f32)[
                127 - np.arange(128)[:, None] + np.arange(15 * 128)[None, :]
            ].transpose(0, 2, 1)).astype(mybir.dt.np(BF16)),
        "w1": inputs["ff_w1"].astype(f32).astype(mybir.dt.np(BF16)),
        "b1c": b1c.copy(),
        "w2": np.ascontiguousarray(
            inputs["ff_w2"].astype(f32).reshape(2, 128, E).transpose(
                1, 0, 2)).astype(mybir.dt.np(BF16)),
        "b2": inputs["ff_b2"].astype(f32).reshape(E, 1),
        "ow": inputs["out_w"].astype(f32),
        "ob": inputs["out_b"].astype(f32).reshape(NCls, 1),
        "ident": np.eye(128, dtype=f32),
        "lng": np.stack([inputs["attn_ln_g"], inputs["ln1_g"],
                         inputs["ln2_g"]]).astype(f32),
        "lnb": np.stack([inputs["attn_ln_b"], inputs["ln1_b"],
                         inputs["ln2_b"]]).astype(f32),
    }
    x = inputs["x"].astype(f32)  # (B, 1, L)
    per_core = []
    for b in range(B):
        xpad = np.zeros((L + KW - 1,), f32)
        xpad[3:3 + L] = x[b, 0]
        per_core.append({"xpad": xpad, **shared})
    return per_core


_NC_CACHE = {}


def kernel(**inputs) -> np.ndarray:
    from concourse.bass_utils import run_bass_kernel_spmd
    if "nc" not in _NC_CACHE:
        _NC_CACHE["nc"] = build_nc()
    nc = _NC_CACHE["nc"]
    in_maps = host_prep(inputs)
    res = run_bass_kernel_spmd(nc, in_maps, core_ids=list(range(NCORES)))
    out = np.stack([res.results[b]["out"].reshape(NCls) for b in range(B)])
    return out.astype(np.float32)


if __name__ == "__main__":
    import reference
    ins = {k: np.asarray(v) for k, v in reference.setup_inputs().items()}
    got = kernel(**ins)
    exp = np.asarray(reference.reference(**reference.setup_inputs()))
    err = np.abs(got - exp).max() / np.abs(exp).max()
    print("Relative error:", err)
